# revision 1
# baseline (speedup 1.0000x reference)
"""Causal self-attention (D=1024, H=16, S=2048, B=2) on 8 trn2 cores.

Sharding: core i handles batch b = i // 4 and head-group g = i % 4
(4 heads = 256 model dims per group). Each core computes
    y_partial[b,g] = softmax_causal(Q K^T / 8) V  @ Wo[rows of g]
for its 4 heads; the host sums the 4 group partials per batch and adds bo.

Per-core kernel (bf16 matmul operands, fp32 PSUM accumulation):
  phase 0: xT[c] <- DMA-transpose of x columns (bf16 XBAR path)
  phase 1: QT/KT = (Wq/Wk)^T x^T + b (head pairs packed on partitions),
           V [t,d] for 4 heads + a ones column (softmax denominator trick)
  phase 2: per (head, 512-col s-block): scoresT = KT^T QT with causal block
           skipping, exp on ACT (2-tile batches), triangular diag-chunk mask
           via gpsimd affine_select, PV accumulation -> outT [65, s] whose
           row 64 is the denominator; reciprocal + DRAM-bounce broadcast +
           DVE multiply -> normalized A^T, packed into head-pair tiles
  phase 3: y = A Wo with K=128 head-pair accumulation
"""

import sys

sys.path.insert(0, "/opt/trn_rl_repo")

import ml_dtypes
import numpy as np

import concourse.bass as bass
import concourse.mybir as mybir
import concourse.tile as tile
from concourse import bacc

P = 128
S = 2048
D = 1024
NH = 4                    # heads per core
DH = 64                   # head dim
DPC = NH * DH             # model dims per core = 256
N_CT = D // P             # 8 contraction chunks
N_ST = S // P             # 16 t tiles of 128
N_SB = S // 512           # 4 s blocks of 512
F32 = mybir.dt.float32
BF16 = mybir.dt.bfloat16
SCALE = 1.0 / 8.0         # 1/sqrt(64)

AF = mybir.ActivationFunctionType
ALU = mybir.AluOpType


def build_nc(mm_mode: str = "bf16", stop_after: int = 99,
             skip_norm: bool = False) -> bass.Bass:
    nc = _build(mm_mode, stop_after, skip_norm)
    if not nc.is_finalized():
        nc.finalize()
    return nc


def _build(mm_mode: str, stop_after: int, skip_norm: bool) -> bass.Bass:
    assert mm_mode == "bf16"
    nc = bacc.Bacc("TRN2", target_bir_lowering=False, debug=False,
                   num_devices=8)

    x_d = nc.dram_tensor("x", [S, D], BF16, kind="ExternalInput")
    wq_d = nc.dram_tensor("wq", [D, DPC], BF16, kind="ExternalInput")
    wk_d = nc.dram_tensor("wk", [D, DPC], BF16, kind="ExternalInput")
    wv_d = nc.dram_tensor("wv", [D, DPC], BF16, kind="ExternalInput")
    wo_d = nc.dram_tensor("wo", [DPC, D], BF16, kind="ExternalInput")
    bq_d = nc.dram_tensor("bq", [DPC], F32, kind="ExternalInput")
    bk_d = nc.dram_tensor("bk", [DPC], F32, kind="ExternalInput")
    bv_d = nc.dram_tensor("bv", [DPC], F32, kind="ExternalInput")
    y_d = nc.dram_tensor("y", [S, D], F32, kind="ExternalOutput")

    with tile.TileContext(nc) as tc:
        with (
            tc.tile_pool(name="const", bufs=1) as const,
            tc.tile_pool(name="xtp", bufs=1) as xtp,
            tc.tile_pool(name="qkv", bufs=1) as qkv,
            tc.tile_pool(name="atp", bufs=1) as atp,
            tc.tile_pool(name="work", bufs=5) as work,
            tc.tile_pool(name="att", bufs=4) as attw,
            tc.tile_pool(name="denp", bufs=4) as denp,
            tc.tile_pool(name="rbp", bufs=4) as rbp,
            tc.tile_pool(name="dscr", bufs=8, space="DRAM") as dscr,
            tc.tile_pool(name="ps", bufs=2, space="PSUM") as psp,
            tc.tile_pool(name="ppv", bufs=4, space="PSUM") as ppv,
        ):
            # ---- weights / constants ----
            wq_s = const.tile([P, N_CT, DPC], BF16)
            wk_s = const.tile([P, N_CT, DPC], BF16)
            wv_s = const.tile([P, N_CT, DPC], BF16)
            nc.sync.dma_start(wq_s, wq_d.rearrange("(o p) d -> p o d", p=P))
            nc.sync.dma_start(wk_s, wk_d.rearrange("(o p) d -> p o d", p=P))
            nc.sync.dma_start(wv_s, wv_d.rearrange("(o p) d -> p o d", p=P))
            # Wo packed by head pairs: rows 128*dc .. 128*dc+127
            wo_s = const.tile([P, 2, D], BF16)
            nc.sync.dma_start(wo_s, wo_d.rearrange("(dc p) e -> p dc e", p=P))

            bq_s = const.tile([P, 2], F32)
            bk_s = const.tile([P, 2], F32)
            nc.sync.dma_start(bq_s, bq_d.rearrange("(o p) -> p o", p=P))
            nc.sync.dma_start(bk_s, bk_d.rearrange("(o p) -> p o", p=P))
            bv_b = const.tile([P, DPC], F32)
            nc.gpsimd.dma_start(
                out=bv_b, in_=bv_d[:].unsqueeze(0).partition_broadcast(P)
            )

            # ---- phase 0: DMA-transpose x into per-chunk xT tiles ----
            # s-block-major so phase-1 groups for early s-blocks can start
            # while later transposes are still on the XBAR.
            xT = [xtp.tile([P, S], BF16, tag=f"xt{c}", name=f"xt{c}")
                  for c in range(N_CT)]
            for g in range(N_SB):
                for c in range(N_CT):
                    nc.sync.dma_start_transpose(
                        xT[c][:, g * 512:(g + 1) * 512],
                        x_d[g * 512:(g + 1) * 512, c * P:(c + 1) * P])

            if stop_after <= 0:
                ys0 = work.tile([P, D], F32, tag="work")
                nc.vector.tensor_copy(ys0, xT[0][:, 0:1024])
                nc.sync.dma_start(y_d[0:P, :], ys0)
                return nc

            # ---- phase 1: projections ----
            # QT/KT: [128 (head-pair d), dc, s]
            QT = qkv.tile([P, 2, S], BF16)
            KT = qkv.tile([P, 2, S], BF16)
            # V_aug: [t-part, t-chunk, head, 65], col 64 == 1.0
            vaug = qkv.tile([P, N_ST, NH, DH + 1], BF16)
            nc.vector.memset(vaug[:, :, :, DH:DH + 1], 1.0)
            vaug_v = vaug[:, :, :, 0:DH]

            for sb in range(N_SB):
                for dc in range(2):
                    for w_s, b_s, dst in ((wq_s, bq_s, QT), (wk_s, bk_s, KT)):
                        ps = psp.tile([P, 1024], F32, tag="ps")
                        for c in range(N_CT):
                            nc.tensor.matmul(
                                ps[:, 0:512],
                                w_s[:, c, dc * P:(dc + 1) * P],
                                xT[c][:, sb * 512:(sb + 1) * 512],
                                start=(c == 0),
                                stop=(c == N_CT - 1),
                            )
                        # psum -> sbuf (bf16) with per-partition bias add
                        nc.vector.tensor_scalar_add(
                            dst[:, dc, sb * 512:(sb + 1) * 512],
                            ps[:, 0:512],
                            b_s[:, dc:dc + 1],
                        )

            for tt in range(N_ST):
                ps = psp.tile([P, 1024], F32, tag="ps")
                pvs = ps[:, 0:DPC]
                for c in range(N_CT):
                    nc.tensor.matmul(
                        pvs,
                        xT[c][:, tt * P:(tt + 1) * P],
                        wv_s[:, c, :],
                        start=(c == 0),
                        stop=(c == N_CT - 1),
                    )
                nc.vector.tensor_add(
                    vaug_v[:, tt, :, :],
                    pvs.rearrange("p (h u) -> p h u", h=NH),
                    bv_b.rearrange("p (h u) -> p h u", h=NH),
                )

            if stop_after <= 1:
                ys0 = work.tile([P, D], F32, tag="work")
                nc.vector.tensor_copy(ys0, QT[:, 0, 0:1024])
                nc.sync.dma_start(y_d[0:P, :], ys0)
                ys1 = work.tile([P, 3, DH], F32, tag="work")
                nc.vector.tensor_copy(ys1, vaug[:, 0:3, 0, 0:DH])
                nc.sync.dma_start(y_d[P:2 * P, 0:192], ys1)
                return nc

            # AT packed by head pairs: [128, dc, s]
            AT = atp.tile([P, 2, S], BF16)

            # ---- phase 2: attention ----
            # Head pairs (2*dc, 2*dc+1) share each score/exp tile: the two
            # K=64 score matmuls go to PE row-groups 0 and 64 (concurrent).
            for dc in range(2):
                for sb in range(N_SB):
                    pvs2 = [ppv.tile([DH + 1, 512], F32, tag="pv",
                                     name=f"pv{dc}_{sb}_{e}")
                            for e in range(2)]
                    t_cnt = 4 * sb + 4
                    for T in range(t_cnt):
                        k = T - 4 * sb
                        ms = 128 * k if k > 0 else 0
                        sc = psp.tile([P, 2, 512], F32, tag="ps")
                        ex = attw.tile([P, 2, 512], BF16, tag="ex")
                        for e in range(2):  # even/odd head of the pair
                            off = DH * e
                            nc.tensor.matmul(
                                sc[:, e, ms:512],
                                KT[off:off + DH, dc, T * P:(T + 1) * P],
                                QT[off:off + DH, dc,
                                   sb * 512 + ms:(sb + 1) * 512],
                                start=True,
                                stop=True,
                            )
                        nc.scalar.activation(
                            ex[:, :, ms:512], sc[:, :, ms:512],
                            AF.Exp, scale=SCALE,
                        )
                        if k >= 0:  # triangular mask on diagonal chunks
                            nc.gpsimd.affine_select(
                                out=ex[:, :, ms:ms + P],
                                in_=ex[:, :, ms:ms + P],
                                compare_op=ALU.is_ge,
                                fill=0.0,
                                base=0,
                                pattern=[[0, 2], [1, P]],
                                channel_multiplier=-1,
                            )
                        for e in range(2):
                            h = 2 * dc + e
                            nc.tensor.matmul(
                                pvs2[e][:, ms:512],
                                vaug[:, T, h, :],
                                ex[:, e, ms:512],
                                start=(T == 0),
                                stop=(T == t_cnt - 1),
                            )
                    for e in range(2):
                        pv = pvs2[e]
                        if skip_norm:
                            if e == 0:
                                nc.vector.tensor_copy(
                                    AT[0:DH, dc, sb * 512:(sb + 1) * 512],
                                    pv[0:DH, :])
                            continue
                        # normalize: row 64 of pv is the denominator
                        den = denp.tile([DH + 1, 512], F32, name="den")
                        nc.vector.reciprocal(
                            out=den[DH:DH + 1, :], in_=pv[DH:DH + 1, :]
                        )
                        dend = dscr.tile([512], F32, name="dend")
                        nc.gpsimd.dma_start(out=dend, in_=den[DH:DH + 1, :])
                        rb = rbp.tile([DH, 512], F32)
                        nc.gpsimd.dma_start(
                            out=rb,
                            in_=dend[:].unsqueeze(0).partition_broadcast(DH),
                        )
                        if e == 0:
                            nc.vector.tensor_mul(
                                AT[0:DH, dc, sb * 512:(sb + 1) * 512],
                                pv[0:DH, :], rb)
                        else:
                            att = attw.tile([DH, 512], BF16, tag="att")
                            nc.vector.tensor_mul(att, pv[0:DH, :], rb)
                            nc.sync.dma_start(
                                AT[DH:P, dc, sb * 512:(sb + 1) * 512], att)

            if stop_after <= 2:
                ys0 = work.tile([DH, S], F32, tag="work")
                nc.vector.tensor_copy(ys0, AT[0:DH, 0, :])
                nc.sync.dma_start(y_d[0:DH, 0:1024], ys0[:, 0:1024])
                nc.sync.dma_start(y_d[DH:2 * DH, 0:1024], ys0[:, 1024:2048])
                return nc

            # ---- phase 3: output projection (K=128 head pairs) ----
            for st in range(N_ST):
                for eb in range(2):
                    ps = psp.tile([P, 1024], F32, tag="ps")
                    for dc in range(2):
                        nc.tensor.matmul(
                            ps[:, 0:512],
                            AT[:, dc, st * P:(st + 1) * P],
                            wo_s[:, dc, eb * 512:(eb + 1) * 512],
                            start=(dc == 0),
                            stop=(dc == 1),
                        )
                    ys = work.tile([P, 512], F32, tag="work")
                    nc.scalar.copy(ys, ps[:, 0:512])
                    nc.sync.dma_start(
                        y_d[st * P:(st + 1) * P, eb * 512:(eb + 1) * 512], ys
                    )

    return nc


_NC_CACHE = {}


def _get_nc(mm_mode="bf16"):
    if mm_mode not in _NC_CACHE:
        _NC_CACHE[mm_mode] = build_nc(mm_mode=mm_mode)
    return _NC_CACHE[mm_mode]


MM_MODE = "bf16"


def make_in_maps(x, Wq, bq, Wk, bk, Wv, bv, Wo, mm_mode=None):
    """Per-core input dicts: core i -> (batch i//4, head-group i%4)."""
    bf = ml_dtypes.bfloat16
    in_maps = []
    for core in range(8):
        b, g = core // 4, core % 4
        sl = slice(g * DPC, (g + 1) * DPC)
        in_maps.append({
            "x": np.ascontiguousarray(x[b]).astype(bf),
            "wq": np.ascontiguousarray(Wq[:, sl]).astype(bf),
            "wk": np.ascontiguousarray(Wk[:, sl]).astype(bf),
            "wv": np.ascontiguousarray(Wv[:, sl]).astype(bf),
            "wo": np.ascontiguousarray(Wo[sl, :]).astype(bf),
            "bq": np.ascontiguousarray(bq[sl]).astype(np.float32),
            "bk": np.ascontiguousarray(bk[sl]).astype(np.float32),
            "bv": np.ascontiguousarray(bv[sl]).astype(np.float32),
        })
    return in_maps


def combine_results(results, bo):
    out = np.zeros((2, S, D), dtype=np.float32)
    for core in range(8):
        out[core // 4] += results[core]["y"]
    out += bo.astype(np.float32)
    return out


_RUNNER_CACHE = {}


def get_runner(mm_mode=None):
    """Build (once) a jitted 8-core runner; returns fn(in_maps) -> results."""
    mode = mm_mode or MM_MODE
    if mode in _RUNNER_CACHE:
        return _RUNNER_CACHE[mode]

    import jax
    from jax.sharding import Mesh, PartitionSpec
    from jax.experimental.shard_map import shard_map
    from concourse import bass2jax, mybir as _mb

    nc = _get_nc(mode)
    bass2jax.install_neuronx_cc_hook()

    pname = nc.partition_id_tensor.name if nc.partition_id_tensor else None
    in_names, out_names, out_avals = [], [], []
    for alloc in nc.m.functions[0].allocations:
        if not isinstance(alloc, _mb.MemoryLocationSet):
            continue
        name = alloc.memorylocations[0].name
        if alloc.kind == "ExternalInput":
            if name != pname:
                in_names.append(name)
        elif alloc.kind == "ExternalOutput":
            out_names.append(name)
            out_avals.append(jax.core.ShapedArray(
                tuple(alloc.tensor_shape), _mb.dt.np(alloc.dtype)))
    n_params = len(in_names)
    all_names = in_names + out_names
    if pname is not None:
        all_names = all_names + [pname]

    def _body(*args):
        operands = list(args)
        if pname is not None:
            operands.append(bass2jax.partition_id_tensor())
        outs = bass2jax._bass_exec_p.bind(
            *operands,
            out_avals=tuple(out_avals),
            in_names=tuple(all_names),
            out_names=tuple(out_names),
            lowering_input_output_aliases=(),
            sim_require_finite=True,
            sim_require_nnan=True,
            nc=nc,
        )
        return tuple(outs)

    devices = jax.devices()[:8]
    mesh = Mesh(np.asarray(devices), ("core",))
    sharded = jax.jit(
        shard_map(_body, mesh=mesh,
                  in_specs=(PartitionSpec("core"),) * (n_params + len(out_names)),
                  out_specs=(PartitionSpec("core"),) * len(out_names),
                  check_rep=False),
        keep_unused=True,
    )

    from jax.sharding import NamedSharding
    zero_outs = [
        jax.device_put(
            np.zeros((8 * a.shape[0], *a.shape[1:]), a.dtype),
            NamedSharding(mesh, PartitionSpec("core")),
        )
        for a in out_avals
    ]

    def run(in_maps):
        concat_in = [
            np.concatenate([np.asarray(m[name]) for m in in_maps], axis=0)
            for name in in_names
        ]
        out_arrs = sharded(*concat_in, *zero_outs)
        return [
            {name: np.asarray(out_arrs[i]).reshape(8, *out_avals[i].shape)[c]
             for i, name in enumerate(out_names)}
            for c in range(8)
        ]

    run.sharded = sharded
    run.in_names = in_names
    run.out_names = out_names
    run.out_avals = out_avals
    run.zero_outs = zero_outs
    _RUNNER_CACHE[mode] = run
    return run


def kernel(x, Wq, bq, Wk, bk, Wv, bv, Wo, bo, **_ignored):
    x = np.asarray(x, dtype=np.float32)
    in_maps = make_in_maps(
        x,
        np.asarray(Wq, np.float32), np.asarray(bq, np.float32),
        np.asarray(Wk, np.float32), np.asarray(bk, np.float32),
        np.asarray(Wv, np.float32), np.asarray(bv, np.float32),
        np.asarray(Wo, np.float32),
    )
    try:
        results = get_runner(MM_MODE)(in_maps)
    except Exception:
        # fallback: stock SPMD runner (slower dispatch, same NEFF)
        from concourse.bass_utils import run_bass_kernel_spmd
        results = run_bass_kernel_spmd(
            _get_nc(MM_MODE), in_maps, core_ids=list(range(8))).results
    return combine_results(results, np.asarray(bo, np.float32))



# revision 9
# speedup vs baseline: 352.3397x; 352.3397x over previous
"""Causal self-attention (D=1024, H=16, S=2048, B=2) on 8 trn2 cores.

Sharding: core i handles batch b = i // 4 and head-group g = i % 4
(4 heads = 256 model dims per group). Each core computes
    y_partial[b,g] = softmax_causal(Q K^T / 8) V  @ Wo[rows of g]
for its 4 heads; the host sums the 4 group partials per batch and adds bo.

Per-core kernel (bf16 matmul operands, fp32 PSUM accumulation), emitted
as a software pipeline over 512-column s-blocks:
  per sb: QT/KT = (Wq/Wk)^T x^T + b (ACT Identity-with-bias psum moves),
          V tiles (DVE bias adds), then attention for both head pairs:
          scoresT = KT^T QT on PE row-group pairs, exp on ACT, triangular
          diag masks on Pool, PV accumulation with the softmax-denominator
          ones-column trick. Head e=0 packs V|1 (denom at psum row 64),
          head e=1 packs 1|V and lands at psum partitions 63..127 so both
          heads normalize lane-aligned: DVE fast-reciprocal of the denom
          row, Pool partition_broadcast, DVE multiply -> AT bf16.
          Output projection y = A Wo trails one s-block behind.
DMA queues: x DMA-transposes split across sync+tensor queues; weights on
gpsimd; y stores on gpsimd/sync (cheap DGE issue paths).
"""

import sys

sys.path.insert(0, "/opt/trn_rl_repo")

import ml_dtypes
import numpy as np

import concourse.bass as bass
import concourse.mybir as mybir
import concourse.tile as tile
from concourse import bacc

P = 128
S = 2048
D = 1024
NH = 4                    # heads per core
DH = 64                   # head dim
DPC = NH * DH             # model dims per core = 256
N_CT = D // P             # 8 contraction chunks
N_ST = S // P             # 16 t tiles of 128
N_SB = S // 512           # 4 s blocks of 512
F32 = mybir.dt.float32
BF16 = mybir.dt.bfloat16
SCALE = 1.0 / 8.0         # 1/sqrt(64)

AF = mybir.ActivationFunctionType
ALU = mybir.AluOpType


def build_nc(mm_mode: str = "bf16", stop_after: int = 99,
             skip_norm: bool = False) -> bass.Bass:
    nc = _build(mm_mode, stop_after, skip_norm)
    if not nc.is_finalized():
        nc.finalize()
    return nc


def _build(mm_mode: str, stop_after: int, skip_norm: bool) -> bass.Bass:
    assert mm_mode == "bf16"
    nc = bacc.Bacc("TRN2", target_bir_lowering=False, debug=False,
                   num_devices=8)

    x_d = nc.dram_tensor("x", [S, D], BF16, kind="ExternalInput")
    wq_d = nc.dram_tensor("wq", [D, DPC], BF16, kind="ExternalInput")
    wk_d = nc.dram_tensor("wk", [D, DPC], BF16, kind="ExternalInput")
    wv_d = nc.dram_tensor("wv", [D, DPC], BF16, kind="ExternalInput")
    wo_d = nc.dram_tensor("wo", [DPC, D], BF16, kind="ExternalInput")
    bq_d = nc.dram_tensor("bq", [DPC], F32, kind="ExternalInput")
    bk_d = nc.dram_tensor("bk", [DPC], F32, kind="ExternalInput")
    bv_d = nc.dram_tensor("bv", [DPC], F32, kind="ExternalInput")
    y_d = nc.dram_tensor("y", [S, D], F32, kind="ExternalOutput")

    with tile.TileContext(nc) as tc:
        with (
            tc.tile_pool(name="const", bufs=1) as const,
            tc.tile_pool(name="xtp", bufs=1) as xtp,
            tc.tile_pool(name="qkv", bufs=1) as qkv,
            tc.tile_pool(name="atp", bufs=1) as atp,
            tc.tile_pool(name="exw", bufs=3) as exw,
            tc.tile_pool(name="rcpw", bufs=2) as rcpw,
            tc.tile_pool(name="bcw", bufs=2) as bcw,
            tc.tile_pool(name="ysp", bufs=4) as ysp,
            tc.tile_pool(name="psA", bufs=2, space="PSUM") as psA,
            tc.tile_pool(name="psc", bufs=2, space="PSUM") as psc,
            tc.tile_pool(name="ppv", bufs=2, space="PSUM") as ppv,
        ):
            # ---- weights / constants (gpsimd DMA queue: cheap issue) ----
            wq_s = const.tile([P, N_CT, DPC], BF16)
            wk_s = const.tile([P, N_CT, DPC], BF16)
            wv_s = const.tile([P, N_CT, DPC], BF16)
            nc.gpsimd.dma_start(wq_s, wq_d.rearrange("(o p) d -> p o d", p=P))
            nc.gpsimd.dma_start(wk_s, wk_d.rearrange("(o p) d -> p o d", p=P))
            nc.gpsimd.dma_start(wv_s, wv_d.rearrange("(o p) d -> p o d", p=P))
            # Wo packed by head pairs: rows 128*dc .. 128*dc+127
            wo_s = const.tile([P, 2, D], BF16)
            nc.gpsimd.dma_start(wo_s, wo_d.rearrange("(dc p) e -> p dc e", p=P))

            bq_s = const.tile([P, 2], F32)
            bk_s = const.tile([P, 2], F32)
            nc.gpsimd.dma_start(bq_s, bq_d.rearrange("(o p) -> p o", p=P))
            nc.gpsimd.dma_start(bk_s, bk_d.rearrange("(o p) -> p o", p=P))
            bv_b = const.tile([P, DPC], F32)
            nc.gpsimd.dma_start(
                out=bv_b, in_=bv_d[:].unsqueeze(0).partition_broadcast(P)
            )
            # bf16 ones row at partition 64: K=1 stationary for the
            # denominator-broadcast matmuls
            ones_s = const.tile([P, DH], BF16)
            nc.vector.memset(ones_s, 1.0)

            # ---- phase 0: DMA-transpose x into per-chunk xT tiles ----
            # s-block-major. Only HWDGE queues (sync, scalar) can transpose;
            # block 0 is start-critical so it splits across both (ACT is
            # idle then), later blocks stay on sync to keep ACT for exp.
            xT = [xtp.tile([P, S], BF16, tag=f"xt{c}", name=f"xt{c}")
                  for c in range(N_CT)]
            for g in range(N_SB):
                for c in range(N_CT):
                    q = nc.scalar if (g == 0 and c % 2 == 1) else nc.sync
                    q.dma_start_transpose(
                        xT[c][:, g * 512:(g + 1) * 512],
                        x_d[g * 512:(g + 1) * 512, c * P:(c + 1) * P])

            # QT/KT: [128 (head-pair d), dc, s]
            QT = qkv.tile([P, 2, S], BF16)
            KT = qkv.tile([P, 2, S], BF16)
            # V_aug: [t-part, t-chunk, head, 65], col 64 == 1.0 so the PV
            # matmul's psum row 64 accumulates the softmax denominator.
            vaug = qkv.tile([P, N_ST, NH, DH + 1], BF16)
            nc.vector.memset(vaug[:, :, :, DH:DH + 1], 1.0)

            # AT packed by head pairs: [128, dc, s]
            AT = atp.tile([P, 2, S], BF16)

            def emit_qkv_proj(sb):
                for dc in range(2):
                    for w_s, b_s, dst in ((wq_s, bq_s, QT), (wk_s, bk_s, KT)):
                        ps = psA.tile([P, 512], F32, tag="psA")
                        for c in range(N_CT):
                            nc.tensor.matmul(
                                ps,
                                w_s[:, c, dc * P:(dc + 1) * P],
                                xT[c][:, sb * 512:(sb + 1) * 512],
                                start=(c == 0),
                                stop=(c == N_CT - 1),
                            )
                        # psum -> sbuf (bf16) with per-partition bias on ACT
                        nc.scalar.activation(
                            dst[:, dc, sb * 512:(sb + 1) * 512], ps,
                            AF.Identity, bias=b_s[:, dc:dc + 1], scale=1.0,
                        )
                for tt in range(4 * sb, 4 * sb + 4):
                    ps = psA.tile([P, 512], F32, tag="psA")
                    pvs = ps[:, 0:DPC]
                    for c in range(N_CT):
                        nc.tensor.matmul(
                            pvs,
                            xT[c][:, tt * P:(tt + 1) * P],
                            wv_s[:, c, :],
                            start=(c == 0),
                            stop=(c == N_CT - 1),
                        )
                    nc.vector.tensor_add(
                        vaug[:, tt, :, 0:DH],
                        pvs.rearrange("p (h u) -> p h u", h=NH),
                        bv_b.rearrange("p (h u) -> p h u", h=NH))

            def emit_attention(sb):
                # Head pairs (2*dc, 2*dc+1) share each score/exp tile: the two
                # K=64 score matmuls go to PE row-groups 0 and 64 (concurrent).
                t_cnt = 4 * sb + 4
                for dc in range(2):
                    pvt = [ppv.tile([P, 512], F32, tag="pv",
                                    name=f"pv{sb}_{dc}_{e}")
                           for e in range(2)]
                    # both heads: rows 0..63 = values, row 64 = denominator
                    pv_dst = (pvt[0][0:DH + 1, :], pvt[1][0:DH + 1, :])
                    pend = []  # deferred PV emission: (T, ms, ex)
                    for T in range(t_cnt):
                        k = T - 4 * sb
                        ms = 128 * k if k > 0 else 0
                        sc = psc.tile([P, 2, 512], F32, tag="sc")
                        ex = exw.tile([P, 2, 512], BF16, tag="ex")
                        for e in range(2):  # even/odd head of the pair
                            off = DH * e
                            nc.tensor.matmul(
                                sc[:, e, ms:512],
                                KT[off:off + DH, dc, T * P:(T + 1) * P],
                                QT[off:off + DH, dc,
                                   sb * 512 + ms:(sb + 1) * 512],
                                start=True,
                                stop=True,
                            )
                        nc.scalar.activation(
                            ex[:, :, ms:512], sc[:, :, ms:512],
                            AF.Exp, scale=SCALE,
                        )
                        if k >= 0:  # triangular mask on diagonal chunks
                            nc.gpsimd.affine_select(
                                out=ex[:, :, ms:ms + P],
                                in_=ex[:, :, ms:ms + P],
                                compare_op=ALU.is_ge,
                                fill=0.0,
                                base=0,
                                pattern=[[0, 2], [1, P]],
                                channel_multiplier=-1,
                            )
                        pend.append((T, ms, ex))
                        if len(pend) > 1:
                            _emit_pv(dc, pv_dst, pend.pop(0), t_cnt)
                    _emit_pv(dc, pv_dst, pend.pop(0), t_cnt)

                    if skip_norm:
                        nc.vector.tensor_copy(
                            AT[0:DH, dc, sb * 512:(sb + 1) * 512],
                            pvt[0][0:DH, :])
                        continue
                    # normalize: ACT-copy denom rows to bf16, PE ones-matmul
                    # broadcasts them to 64 psum partitions, one fast-recip
                    # over both heads, DVE multiplies into AT (e=1 via DMA)
                    drow = rcpw.tile([P, 2, 512], BF16, tag="rcp")
                    bcp = psc.tile([P, 2, 512], F32, tag="sc")
                    for e in range(2):
                        nc.scalar.activation(
                            drow[DH:DH + 1, e, :], pvt[e][DH:DH + 1, :],
                            AF.Copy)
                        nc.tensor.matmul(
                            bcp[0:DH, e, :], ones_s[DH:DH + 1, :],
                            drow[DH:DH + 1, e, :], start=True, stop=True)
                    rcb = bcw.tile([P, 2, 512], F32, tag="bc")
                    nc.vector.reciprocal_approx_fast(
                        out=rcb[0:DH, :, :], in_=bcp[0:DH, :, :])
                    nc.vector.tensor_mul(
                        AT[0:DH, dc, sb * 512:(sb + 1) * 512],
                        pvt[0][0:DH, :], rcb[0:DH, 0, :])
                    att = exw.tile([DH, 512], BF16, tag="att")
                    nc.vector.tensor_mul(att, pvt[1][0:DH, :], rcb[0:DH, 1, :])
                    nc.gpsimd.dma_start(
                        AT[DH:P, dc, sb * 512:(sb + 1) * 512], att)

            def _emit_pv(dc, pv_dst, item, t_cnt):
                T, ms, ex = item
                for e in range(2):
                    h = 2 * dc + e
                    nc.tensor.matmul(
                        pv_dst[e][:, ms:512],
                        vaug[:, T, h, :],
                        ex[:, e, ms:512],
                        start=(T == 0),
                        stop=(T == t_cnt - 1),
                    )

            def emit_outproj(sb):
                for i, st in enumerate(range(4 * sb, 4 * sb + 4)):
                    for eb in range(2):
                        ps = psA.tile([P, 512], F32, tag="psA")
                        for dc in range(2):
                            nc.tensor.matmul(
                                ps,
                                AT[:, dc, st * P:(st + 1) * P],
                                wo_s[:, dc, eb * 512:(eb + 1) * 512],
                                start=(dc == 0),
                                stop=(dc == 1),
                            )
                        ys = ysp.tile([P, 512], F32, tag="ys")
                        if eb == 0:
                            nc.scalar.activation(ys, ps, AF.Copy)
                        else:
                            nc.vector.tensor_copy(ys, ps)
                        q = nc.gpsimd if (eb == 0) else nc.sync
                        q.dma_start(
                            y_d[st * P:(st + 1) * P,
                                eb * 512:(eb + 1) * 512], ys)

            for sb in range(N_SB):
                emit_qkv_proj(sb)
                if sb >= 1:
                    emit_outproj(sb - 1)
                if stop_after <= 1 and sb == 0:
                    break
                emit_attention(sb)
            if stop_after > 1:
                emit_outproj(N_SB - 1)

    return nc


_NC_CACHE = {}


def _get_nc(mm_mode="bf16"):
    if mm_mode not in _NC_CACHE:
        _NC_CACHE[mm_mode] = build_nc(mm_mode=mm_mode)
    return _NC_CACHE[mm_mode]


MM_MODE = "bf16"


def make_in_maps(x, Wq, bq, Wk, bk, Wv, bv, Wo, mm_mode=None):
    """Per-core input dicts: core i -> (batch i//4, head-group i%4)."""
    bf = ml_dtypes.bfloat16
    in_maps = []
    for core in range(8):
        b, g = core // 4, core % 4
        sl = slice(g * DPC, (g + 1) * DPC)
        in_maps.append({
            "x": np.ascontiguousarray(x[b]).astype(bf),
            "wq": np.ascontiguousarray(Wq[:, sl]).astype(bf),
            "wk": np.ascontiguousarray(Wk[:, sl]).astype(bf),
            "wv": np.ascontiguousarray(Wv[:, sl]).astype(bf),
            "wo": np.ascontiguousarray(Wo[sl, :]).astype(bf),
            "bq": np.ascontiguousarray(bq[sl]).astype(np.float32),
            "bk": np.ascontiguousarray(bk[sl]).astype(np.float32),
            "bv": np.ascontiguousarray(bv[sl]).astype(np.float32),
        })
    return in_maps


def combine_results(results, bo):
    out = np.zeros((2, S, D), dtype=np.float32)
    for core in range(8):
        out[core // 4] += results[core]["y"]
    out += bo.astype(np.float32)
    return out


_RUNNER_CACHE = {}


def get_runner(mm_mode=None):
    """Build (once) a jitted 8-core runner; returns fn(in_maps) -> results."""
    mode = mm_mode or MM_MODE
    if mode in _RUNNER_CACHE:
        return _RUNNER_CACHE[mode]

    import jax
    from jax.sharding import Mesh, PartitionSpec
    from jax.experimental.shard_map import shard_map
    from concourse import bass2jax, mybir as _mb

    nc = _get_nc(mode)
    bass2jax.install_neuronx_cc_hook()

    pname = nc.partition_id_tensor.name if nc.partition_id_tensor else None
    in_names, out_names, out_avals = [], [], []
    for alloc in nc.m.functions[0].allocations:
        if not isinstance(alloc, _mb.MemoryLocationSet):
            continue
        name = alloc.memorylocations[0].name
        if alloc.kind == "ExternalInput":
            if name != pname:
                in_names.append(name)
        elif alloc.kind == "ExternalOutput":
            out_names.append(name)
            out_avals.append(jax.core.ShapedArray(
                tuple(alloc.tensor_shape), _mb.dt.np(alloc.dtype)))
    n_params = len(in_names)
    all_names = in_names + out_names
    if pname is not None:
        all_names = all_names + [pname]

    def _body(*args):
        operands = list(args)
        if pname is not None:
            operands.append(bass2jax.partition_id_tensor())
        outs = bass2jax._bass_exec_p.bind(
            *operands,
            out_avals=tuple(out_avals),
            in_names=tuple(all_names),
            out_names=tuple(out_names),
            lowering_input_output_aliases=(),
            sim_require_finite=True,
            sim_require_nnan=True,
            nc=nc,
        )
        return tuple(outs)

    devices = jax.devices()[:8]
    mesh = Mesh(np.asarray(devices), ("core",))
    sharded = jax.jit(
        shard_map(_body, mesh=mesh,
                  in_specs=(PartitionSpec("core"),) * (n_params + len(out_names)),
                  out_specs=(PartitionSpec("core"),) * len(out_names),
                  check_rep=False),
        keep_unused=True,
    )

    from jax.sharding import NamedSharding
    zero_outs = [
        jax.device_put(
            np.zeros((8 * a.shape[0], *a.shape[1:]), a.dtype),
            NamedSharding(mesh, PartitionSpec("core")),
        )
        for a in out_avals
    ]

    def run(in_maps):
        concat_in = [
            np.concatenate([np.asarray(m[name]) for m in in_maps], axis=0)
            for name in in_names
        ]
        out_arrs = sharded(*concat_in, *zero_outs)
        return [
            {name: np.asarray(out_arrs[i]).reshape(8, *out_avals[i].shape)[c]
             for i, name in enumerate(out_names)}
            for c in range(8)
        ]

    run.sharded = sharded
    run.in_names = in_names
    run.out_names = out_names
    run.out_avals = out_avals
    run.zero_outs = zero_outs
    _RUNNER_CACHE[mode] = run
    return run


def kernel(x, Wq, bq, Wk, bk, Wv, bv, Wo, bo, **_ignored):
    x = np.asarray(x, dtype=np.float32)
    in_maps = make_in_maps(
        x,
        np.asarray(Wq, np.float32), np.asarray(bq, np.float32),
        np.asarray(Wk, np.float32), np.asarray(bk, np.float32),
        np.asarray(Wv, np.float32), np.asarray(bv, np.float32),
        np.asarray(Wo, np.float32),
    )
    try:
        results = get_runner(MM_MODE)(in_maps)
    except Exception:
        # fallback: stock SPMD runner (slower dispatch, same NEFF)
        from concourse.bass_utils import run_bass_kernel_spmd
        results = run_bass_kernel_spmd(
            _get_nc(MM_MODE), in_maps, core_ids=list(range(8))).results
    return combine_results(results, np.asarray(bo, np.float32))


# revision 14
# speedup vs baseline: 353.9771x; 1.0046x over previous
"""Causal self-attention (D=1024, H=16, S=2048, B=2) on 8 trn2 cores.

Sharding: core i handles batch b = i // 4 and head-group g = i % 4
(4 heads = 256 model dims per group). Each core computes
    y_partial[b,g] = softmax_causal(Q K^T / 8) V  @ Wo[rows of g]
for its 4 heads; the host sums the 4 group partials per batch and adds bo.

Per-core kernel (bf16 matmul operands, fp32 PSUM accumulation), emitted
as a software pipeline over 512-column s-blocks:
  per sb: QT/KT = (Wq/Wk)^T x^T + b (ACT Identity-with-bias psum moves),
          V tiles (DVE bias adds), then attention for both head pairs:
          scoresT = KT^T QT on PE row-group pairs, exp on ACT, triangular
          diag masks on Pool, PV accumulation with the softmax-denominator
          ones-column trick. Head e=0 packs V|1 (denom at psum row 64),
          head e=1 packs 1|V and lands at psum partitions 63..127 so both
          heads normalize lane-aligned: DVE fast-reciprocal of the denom
          row, Pool partition_broadcast, DVE multiply -> AT bf16.
          Output projection y = A Wo trails one s-block behind.
DMA queues: x DMA-transposes split across sync+tensor queues; weights on
gpsimd; y stores on gpsimd/sync (cheap DGE issue paths).
"""

import sys

sys.path.insert(0, "/opt/trn_rl_repo")

import ml_dtypes
import numpy as np

import concourse.bass as bass
import concourse.mybir as mybir
import concourse.tile as tile
from concourse import bacc

P = 128
S = 2048
D = 1024
NH = 4                    # heads per core
DH = 64                   # head dim
DPC = NH * DH             # model dims per core = 256
N_CT = D // P             # 8 contraction chunks
N_ST = S // P             # 16 t tiles of 128
N_SB = S // 512           # 4 s blocks of 512
F32 = mybir.dt.float32
BF16 = mybir.dt.bfloat16
SCALE = 1.0 / 8.0         # 1/sqrt(64)

AF = mybir.ActivationFunctionType
ALU = mybir.AluOpType


def build_nc(mm_mode: str = "bf16", stop_after: int = 99,
             skip_norm: bool = False) -> bass.Bass:
    nc = _build(mm_mode, stop_after, skip_norm)
    if not nc.is_finalized():
        nc.finalize()
    return nc


def _build(mm_mode: str, stop_after: int, skip_norm: bool) -> bass.Bass:
    assert mm_mode == "bf16"
    nc = bacc.Bacc("TRN2", target_bir_lowering=False, debug=False,
                   num_devices=8)

    x_d = nc.dram_tensor("x", [S, D], BF16, kind="ExternalInput")
    wq_d = nc.dram_tensor("wq", [D, DPC], BF16, kind="ExternalInput")
    wk_d = nc.dram_tensor("wk", [D, DPC], BF16, kind="ExternalInput")
    wv_d = nc.dram_tensor("wv", [D, DPC], BF16, kind="ExternalInput")
    wo_d = nc.dram_tensor("wo", [DPC, D], BF16, kind="ExternalInput")
    bq_d = nc.dram_tensor("bq", [DPC], F32, kind="ExternalInput")
    bk_d = nc.dram_tensor("bk", [DPC], F32, kind="ExternalInput")
    bv_d = nc.dram_tensor("bv", [DPC], F32, kind="ExternalInput")
    y_d = nc.dram_tensor("y", [S, D], F32, kind="ExternalOutput")

    with tile.TileContext(nc) as tc:
        with (
            tc.tile_pool(name="const", bufs=1) as const,
            tc.tile_pool(name="xtp", bufs=1) as xtp,
            tc.tile_pool(name="qkv", bufs=1) as qkv,
            tc.tile_pool(name="atp", bufs=1) as atp,
            tc.tile_pool(name="exw", bufs=3) as exw,
            tc.tile_pool(name="rcpw", bufs=2) as rcpw,
            tc.tile_pool(name="bcw", bufs=2) as bcw,
            tc.tile_pool(name="ysp", bufs=4) as ysp,
            tc.tile_pool(name="psA", bufs=2, space="PSUM") as psA,
            tc.tile_pool(name="psc", bufs=2, space="PSUM") as psc,
            tc.tile_pool(name="ppv", bufs=2, space="PSUM") as ppv,
        ):
            # ---- weights / constants: sync HWDGE queue, ahead of the x
            # transposes so nothing latency-critical rides the SWDGE path ----
            wq_s = const.tile([P, N_CT, DPC], BF16)
            wk_s = const.tile([P, N_CT, DPC], BF16)
            wv_s = const.tile([P, N_CT, DPC], BF16)
            nc.sync.dma_start(wq_s, wq_d.rearrange("(o p) d -> p o d", p=P))
            nc.sync.dma_start(wk_s, wk_d.rearrange("(o p) d -> p o d", p=P))
            nc.sync.dma_start(wv_s, wv_d.rearrange("(o p) d -> p o d", p=P))
            # Wo packed by head pairs: rows 128*dc .. 128*dc+127
            wo_s = const.tile([P, 2, D], BF16)
            nc.sync.dma_start(wo_s, wo_d.rearrange("(dc p) e -> p dc e", p=P))

            bq_s = const.tile([P, 2], F32)
            bk_s = const.tile([P, 2], F32)
            nc.sync.dma_start(bq_s, bq_d.rearrange("(o p) -> p o", p=P))
            nc.sync.dma_start(bk_s, bk_d.rearrange("(o p) -> p o", p=P))
            # bv as a single bf16 row: added into the V psum via a K=1
            # ones-stationary matmul instead of a broadcast DMA
            bv_f32 = const.tile([1, DPC], F32)
            nc.sync.dma_start(bv_f32, bv_d[:].unsqueeze(0))
            bv_row = const.tile([1, DPC], BF16)
            nc.vector.tensor_copy(bv_row, bv_f32)
            # bf16 ones: K=1 stationary rows for broadcast matmuls
            ones_s = const.tile([P, P], BF16)
            nc.vector.memset(ones_s, 1.0)

            # ---- phase 0: DMA-transpose x into per-chunk xT tiles ----
            # s-block-major. Only HWDGE queues (sync, scalar) can transpose.
            # Block 0 (start-critical) + half of block 1 ride the scalar
            # queue (ACT is idle then); the rest follow the weight loads on
            # sync so exp keeps the ACT sequencer afterwards.
            xT = [xtp.tile([P, S], BF16, tag=f"xt{c}", name=f"xt{c}")
                  for c in range(N_CT)]
            for g in range(N_SB):
                for c in range(N_CT):
                    q = nc.scalar if (g == 0 or (g == 1 and c % 2 == 1)) \
                        else nc.sync
                    q.dma_start_transpose(
                        xT[c][:, g * 512:(g + 1) * 512],
                        x_d[g * 512:(g + 1) * 512, c * P:(c + 1) * P])

            # QT/KT: [128 (head-pair d), dc, s]
            QT = qkv.tile([P, 2, S], BF16)
            KT = qkv.tile([P, 2, S], BF16)
            # V_aug: [t-part, t-chunk, head, 65], col 64 == 1.0 so the PV
            # matmul's psum row 64 accumulates the softmax denominator.
            vaug = qkv.tile([P, N_ST, NH, DH + 1], BF16)
            nc.vector.memset(vaug[:, :, :, DH:DH + 1], 1.0)

            # AT packed by head pairs: [128, dc, s]
            AT = atp.tile([P, 2, S], BF16)

            def emit_qkv_proj(sb):
                for dc in range(2):
                    for w_s, b_s, dst in ((wq_s, bq_s, QT), (wk_s, bk_s, KT)):
                        ps = psA.tile([P, 512], F32, tag="psA")
                        for c in range(N_CT):
                            nc.tensor.matmul(
                                ps,
                                w_s[:, c, dc * P:(dc + 1) * P],
                                xT[c][:, sb * 512:(sb + 1) * 512],
                                start=(c == 0),
                                stop=(c == N_CT - 1),
                            )
                        # psum -> sbuf (bf16) with per-partition bias on ACT
                        nc.scalar.activation(
                            dst[:, dc, sb * 512:(sb + 1) * 512], ps,
                            AF.Identity, bias=b_s[:, dc:dc + 1], scale=1.0,
                        )
                for tt in range(4 * sb, 4 * sb + 4):
                    ps = psA.tile([P, 512], F32, tag="psA")
                    pvs = ps[:, 0:DPC]
                    # K=1 ones matmul seeds the psum with bv broadcast
                    nc.tensor.matmul(ps[:, 0:DPC], ones_s[0:1, :], bv_row,
                                     start=True, stop=False)
                    for c in range(N_CT):
                        nc.tensor.matmul(
                            pvs,
                            xT[c][:, tt * P:(tt + 1) * P],
                            wv_s[:, c, :],
                            start=False,
                            stop=(c == N_CT - 1),
                        )
                    nc.vector.tensor_copy(
                        vaug[:, tt, :, 0:DH],
                        pvs.rearrange("p (h u) -> p h u", h=NH))

            def emit_attention(sb):
                # Head pairs (2*dc, 2*dc+1) share each score/exp tile: the two
                # K=64 score matmuls go to PE row-groups 0 and 64 (concurrent).
                t_cnt = 4 * sb + 4
                for dc in range(2):
                    pvt = [ppv.tile([P, 512], F32, tag="pv",
                                    name=f"pv{sb}_{dc}_{e}")
                           for e in range(2)]
                    # both heads: rows 0..63 = values, row 64 = denominator
                    pv_dst = (pvt[0][0:DH + 1, :], pvt[1][0:DH + 1, :])
                    pend = []  # deferred PV emission: (T, ms, ex)
                    for T in range(t_cnt):
                        k = T - 4 * sb
                        ms = 128 * k if k > 0 else 0
                        sc = psc.tile([P, 2, 512], F32, tag="sc")
                        ex = exw.tile([P, 2, 512], BF16, tag="ex")
                        for e in range(2):  # even/odd head of the pair
                            off = DH * e
                            nc.tensor.matmul(
                                sc[:, e, ms:512],
                                KT[off:off + DH, dc, T * P:(T + 1) * P],
                                QT[off:off + DH, dc,
                                   sb * 512 + ms:(sb + 1) * 512],
                                start=True,
                                stop=True,
                            )
                        nc.scalar.activation(
                            ex[:, :, ms:512], sc[:, :, ms:512],
                            AF.Exp, scale=SCALE,
                        )
                        if k >= 0:  # triangular mask on diagonal chunks
                            nc.gpsimd.affine_select(
                                out=ex[:, :, ms:ms + P],
                                in_=ex[:, :, ms:ms + P],
                                compare_op=ALU.is_ge,
                                fill=0.0,
                                base=0,
                                pattern=[[0, 2], [1, P]],
                                channel_multiplier=-1,
                            )
                        pend.append((T, ms, ex))
                        if len(pend) > 1:
                            _emit_pv(dc, pv_dst, pend.pop(0), t_cnt)
                    _emit_pv(dc, pv_dst, pend.pop(0), t_cnt)

                    if skip_norm:
                        nc.vector.tensor_copy(
                            AT[0:DH, dc, sb * 512:(sb + 1) * 512],
                            pvt[0][0:DH, :])
                        continue
                    # normalize: ACT-copy denom rows to bf16, PE ones-matmul
                    # broadcasts them to 64 psum partitions, one fast-recip
                    # over both heads, DVE multiplies into AT (e=1 via DMA)
                    drow = rcpw.tile([P, 2, 512], BF16, tag="rcp")
                    bcp = psc.tile([P, 2, 512], F32, tag="sc")
                    for e in range(2):
                        nc.scalar.activation(
                            drow[DH:DH + 1, e, :], pvt[e][DH:DH + 1, :],
                            AF.Copy)
                        nc.tensor.matmul(
                            bcp[0:DH, e, :], ones_s[DH:DH + 1, 0:DH],
                            drow[DH:DH + 1, e, :], start=True, stop=True)
                    rcb = bcw.tile([P, 2, 512], F32, tag="bc")
                    nc.vector.reciprocal_approx_fast(
                        out=rcb[0:DH, :, :], in_=bcp[0:DH, :, :])
                    nc.vector.tensor_mul(
                        AT[0:DH, dc, sb * 512:(sb + 1) * 512],
                        pvt[0][0:DH, :], rcb[0:DH, 0, :])
                    att = exw.tile([DH, 512], BF16, tag="att")
                    nc.vector.tensor_mul(att, pvt[1][0:DH, :], rcb[0:DH, 1, :])
                    nc.gpsimd.dma_start(
                        AT[DH:P, dc, sb * 512:(sb + 1) * 512], att)

            def _emit_pv(dc, pv_dst, item, t_cnt):
                T, ms, ex = item
                for e in range(2):
                    h = 2 * dc + e
                    nc.tensor.matmul(
                        pv_dst[e][:, ms:512],
                        vaug[:, T, h, :],
                        ex[:, e, ms:512],
                        start=(T == 0),
                        stop=(T == t_cnt - 1),
                    )

            def emit_outproj(sb):
                for i, st in enumerate(range(4 * sb, 4 * sb + 4)):
                    for eb in range(2):
                        ps = psA.tile([P, 512], F32, tag="psA")
                        for dc in range(2):
                            nc.tensor.matmul(
                                ps,
                                AT[:, dc, st * P:(st + 1) * P],
                                wo_s[:, dc, eb * 512:(eb + 1) * 512],
                                start=(dc == 0),
                                stop=(dc == 1),
                            )
                        ys = ysp.tile([P, 512], F32, tag="ys")
                        if eb == 0:
                            nc.scalar.activation(ys, ps, AF.Copy)
                        else:
                            nc.vector.tensor_copy(ys, ps)
                        q = nc.gpsimd if (eb == 0) else nc.sync
                        q.dma_start(
                            y_d[st * P:(st + 1) * P,
                                eb * 512:(eb + 1) * 512], ys)

            for sb in range(N_SB):
                emit_qkv_proj(sb)
                if sb >= 1:
                    emit_outproj(sb - 1)
                if stop_after <= 1 and sb == 0:
                    break
                emit_attention(sb)
            if stop_after > 1:
                emit_outproj(N_SB - 1)

    return nc


_NC_CACHE = {}


def _get_nc(mm_mode="bf16"):
    if mm_mode not in _NC_CACHE:
        _NC_CACHE[mm_mode] = build_nc(mm_mode=mm_mode)
    return _NC_CACHE[mm_mode]


MM_MODE = "bf16"


def make_in_maps(x, Wq, bq, Wk, bk, Wv, bv, Wo, mm_mode=None):
    """Per-core input dicts: core i -> (batch i//4, head-group i%4)."""
    bf = ml_dtypes.bfloat16
    in_maps = []
    for core in range(8):
        b, g = core // 4, core % 4
        sl = slice(g * DPC, (g + 1) * DPC)
        in_maps.append({
            "x": np.ascontiguousarray(x[b]).astype(bf),
            "wq": np.ascontiguousarray(Wq[:, sl]).astype(bf),
            "wk": np.ascontiguousarray(Wk[:, sl]).astype(bf),
            "wv": np.ascontiguousarray(Wv[:, sl]).astype(bf),
            "wo": np.ascontiguousarray(Wo[sl, :]).astype(bf),
            "bq": np.ascontiguousarray(bq[sl]).astype(np.float32),
            "bk": np.ascontiguousarray(bk[sl]).astype(np.float32),
            "bv": np.ascontiguousarray(bv[sl]).astype(np.float32),
        })
    return in_maps


def combine_results(results, bo):
    out = np.zeros((2, S, D), dtype=np.float32)
    for core in range(8):
        out[core // 4] += results[core]["y"]
    out += bo.astype(np.float32)
    return out


_RUNNER_CACHE = {}


def get_runner(mm_mode=None):
    """Build (once) a jitted 8-core runner; returns fn(in_maps) -> results."""
    mode = mm_mode or MM_MODE
    if mode in _RUNNER_CACHE:
        return _RUNNER_CACHE[mode]

    import jax
    from jax.sharding import Mesh, PartitionSpec
    from jax.experimental.shard_map import shard_map
    from concourse import bass2jax, mybir as _mb

    nc = _get_nc(mode)
    bass2jax.install_neuronx_cc_hook()

    pname = nc.partition_id_tensor.name if nc.partition_id_tensor else None
    in_names, out_names, out_avals = [], [], []
    for alloc in nc.m.functions[0].allocations:
        if not isinstance(alloc, _mb.MemoryLocationSet):
            continue
        name = alloc.memorylocations[0].name
        if alloc.kind == "ExternalInput":
            if name != pname:
                in_names.append(name)
        elif alloc.kind == "ExternalOutput":
            out_names.append(name)
            out_avals.append(jax.core.ShapedArray(
                tuple(alloc.tensor_shape), _mb.dt.np(alloc.dtype)))
    n_params = len(in_names)
    all_names = in_names + out_names
    if pname is not None:
        all_names = all_names + [pname]

    def _body(*args):
        operands = list(args)
        if pname is not None:
            operands.append(bass2jax.partition_id_tensor())
        outs = bass2jax._bass_exec_p.bind(
            *operands,
            out_avals=tuple(out_avals),
            in_names=tuple(all_names),
            out_names=tuple(out_names),
            lowering_input_output_aliases=(),
            sim_require_finite=True,
            sim_require_nnan=True,
            nc=nc,
        )
        return tuple(outs)

    devices = jax.devices()[:8]
    mesh = Mesh(np.asarray(devices), ("core",))
    sharded = jax.jit(
        shard_map(_body, mesh=mesh,
                  in_specs=(PartitionSpec("core"),) * (n_params + len(out_names)),
                  out_specs=(PartitionSpec("core"),) * len(out_names),
                  check_rep=False),
        keep_unused=True,
    )

    from jax.sharding import NamedSharding
    zero_outs = [
        jax.device_put(
            np.zeros((8 * a.shape[0], *a.shape[1:]), a.dtype),
            NamedSharding(mesh, PartitionSpec("core")),
        )
        for a in out_avals
    ]

    def run(in_maps):
        concat_in = [
            np.concatenate([np.asarray(m[name]) for m in in_maps], axis=0)
            for name in in_names
        ]
        out_arrs = sharded(*concat_in, *zero_outs)
        return [
            {name: np.asarray(out_arrs[i]).reshape(8, *out_avals[i].shape)[c]
             for i, name in enumerate(out_names)}
            for c in range(8)
        ]

    run.sharded = sharded
    run.in_names = in_names
    run.out_names = out_names
    run.out_avals = out_avals
    run.zero_outs = zero_outs
    _RUNNER_CACHE[mode] = run
    return run


def kernel(x, Wq, bq, Wk, bk, Wv, bv, Wo, bo, **_ignored):
    x = np.asarray(x, dtype=np.float32)
    in_maps = make_in_maps(
        x,
        np.asarray(Wq, np.float32), np.asarray(bq, np.float32),
        np.asarray(Wk, np.float32), np.asarray(bk, np.float32),
        np.asarray(Wv, np.float32), np.asarray(bv, np.float32),
        np.asarray(Wo, np.float32),
    )
    try:
        results = get_runner(MM_MODE)(in_maps)
    except Exception:
        # fallback: stock SPMD runner (slower dispatch, same NEFF)
        from concourse.bass_utils import run_bass_kernel_spmd
        results = run_bass_kernel_spmd(
            _get_nc(MM_MODE), in_maps, core_ids=list(range(8))).results
    return combine_results(results, np.asarray(bo, np.float32))


# revision 21
# speedup vs baseline: 381.6421x; 1.0782x over previous
"""Causal self-attention (D=1024, H=16, S=2048, B=2) on 8 trn2 cores.

Sharding: core i handles batch b = i // 4 and head-group g = i % 4
(4 heads = 256 model dims per group). Each core computes
    y_partial[b,g] = softmax_causal(Q K^T / 8) V  @ Wo[rows of g]
for its 4 heads; the host sums the 4 group partials per batch and adds bo.

Per-core kernel (bf16 matmul operands, fp32 PSUM accumulation), emitted
as a software pipeline over 512-column s-blocks:
  per sb: QT/KT = (Wq/Wk)^T x^T + b (ACT Identity-with-bias psum moves),
          V tiles (DVE bias adds), then attention for both head pairs:
          scoresT = KT^T QT on PE row-group pairs, exp on ACT, triangular
          diag masks on Pool, PV accumulation with the softmax-denominator
          ones-column trick. Head e=0 packs V|1 (denom at psum row 64),
          head e=1 packs 1|V and lands at psum partitions 63..127 so both
          heads normalize lane-aligned: DVE fast-reciprocal of the denom
          row, Pool partition_broadcast, DVE multiply -> AT bf16.
          Output projection y = A Wo trails one s-block behind.
DMA queues: x DMA-transposes split across sync+tensor queues; weights on
gpsimd; y stores on gpsimd/sync (cheap DGE issue paths).
"""

import sys

sys.path.insert(0, "/opt/trn_rl_repo")

import ml_dtypes
import numpy as np

import concourse.bass as bass
import concourse.mybir as mybir
import concourse.tile as tile
from concourse import bacc

P = 128
S = 2048
D = 1024
NH = 4                    # heads per core
DH = 64                   # head dim
DPC = NH * DH             # model dims per core = 256
N_CT = D // P             # 8 contraction chunks
N_ST = S // P             # 16 t tiles of 128
N_SB = S // 512           # 4 s blocks of 512
F32 = mybir.dt.float32
BF16 = mybir.dt.bfloat16
SCALE = 1.0 / 8.0         # 1/sqrt(64)

AF = mybir.ActivationFunctionType
ALU = mybir.AluOpType


def build_nc(mm_mode: str = "bf16", stop_after: int = 99,
             skip_norm: bool = False) -> bass.Bass:
    nc = _build(mm_mode, stop_after, skip_norm)
    if not nc.is_finalized():
        nc.finalize()
    return nc


def _build(mm_mode: str, stop_after: int, skip_norm: bool) -> bass.Bass:
    assert mm_mode == "bf16"
    nc = bacc.Bacc("TRN2", target_bir_lowering=False, debug=False,
                   num_devices=8)

    x_d = nc.dram_tensor("x", [S, D], BF16, kind="ExternalInput")
    # wq/wk/wv stacked host-side: one DMA instead of three
    wqkv_d = nc.dram_tensor("wqkv", [3 * D, DPC], BF16, kind="ExternalInput")
    wo_d = nc.dram_tensor("wo", [DPC, D], BF16, kind="ExternalInput")
    # bq|bk|bv stacked host-side: one DMA instead of three
    b3_d = nc.dram_tensor("b3", [3 * DPC], F32, kind="ExternalInput")
    y_d = nc.dram_tensor("y", [S, D], F32, kind="ExternalOutput")

    with tile.TileContext(nc) as tc:
        with (
            tc.tile_pool(name="const", bufs=1) as const,
            tc.tile_pool(name="xtp", bufs=1) as xtp,
            tc.tile_pool(name="qkv", bufs=1) as qkv,
            tc.tile_pool(name="atp", bufs=1) as atp,
            tc.tile_pool(name="exw", bufs=3) as exw,
            tc.tile_pool(name="rcpw", bufs=2) as rcpw,
            tc.tile_pool(name="bcw", bufs=2) as bcw,
            tc.tile_pool(name="ysp", bufs=4) as ysp,
            tc.tile_pool(name="psA", bufs=2, space="PSUM") as psA,
            tc.tile_pool(name="psc", bufs=2, space="PSUM") as psc,
            tc.tile_pool(name="ppv", bufs=2, space="PSUM") as ppv,
        ):
            # ---- weights / constants: 3 DMAs total on the sync HWDGE
            # queue, ahead of the x transposes (the DMA semaphore ring
            # serializes across queues, so every early DMA counts) ----
            wqkv_s = const.tile([P, 3 * N_CT, DPC], BF16)
            nc.sync.dma_start(wqkv_s,
                              wqkv_d.rearrange("(o p) d -> p o d", p=P))
            wq_s = wqkv_s[:, 0 * N_CT:1 * N_CT, :]
            wk_s = wqkv_s[:, 1 * N_CT:2 * N_CT, :]
            wv_s = wqkv_s[:, 2 * N_CT:3 * N_CT, :]
            # Wo packed by head pairs: rows 128*dc .. 128*dc+127
            wo_s = const.tile([P, 2, D], BF16)
            nc.sync.dma_start(wo_s, wo_d.rearrange("(dc p) e -> p dc e", p=P))

            # biases as bf16 rows on partition 0; applied by K=1
            # ones-stationary matmuls that seed the projection psums
            b3_f32 = const.tile([1, 3 * DPC], F32)
            nc.sync.dma_start(b3_f32, b3_d[:].unsqueeze(0))
            b3_row = const.tile([1, 3 * DPC], BF16)
            nc.vector.tensor_copy(b3_row, b3_f32)
            bq_row = b3_row[:, 0 * DPC:1 * DPC]
            bk_row = b3_row[:, 1 * DPC:2 * DPC]
            bv_row = b3_row[:, 2 * DPC:3 * DPC]
            # bf16 ones: K=1 stationary / moving rows for broadcasts
            ones_s = const.tile([P, 512], BF16)
            nc.vector.memset(ones_s, 1.0)

            # ---- phase 0: DMA-transpose x into per-chunk xT tiles ----
            # 16 half-column transposes (s 0..1023 then 1024..2047), split
            # across the two HWDGE queues (sync, scalar): fewest DMAs on
            # the serializing semaphore ring while the first half still
            # lands early enough to feed s-blocks 0-1.
            xT = [xtp.tile([P, S], BF16, tag=f"xt{c}", name=f"xt{c}")
                  for c in range(N_CT)]
            for h in range(2):
                for c in range(N_CT):
                    q = nc.scalar if (c % 2 == 1) else nc.sync
                    q.dma_start_transpose(
                        xT[c][:, h * 1024:(h + 1) * 1024],
                        x_d[h * 1024:(h + 1) * 1024, c * P:(c + 1) * P])

            # QT/KT: [128 (head-pair d), dc, s]
            QT = qkv.tile([P, 2, S], BF16)
            KT = qkv.tile([P, 2, S], BF16)
            # V_aug: [t-part, t-chunk, head, 65], col 64 == 1.0 so the PV
            # matmul's psum row 64 accumulates the softmax denominator.
            vaug = qkv.tile([P, N_ST, NH, DH + 1], BF16)
            nc.vector.memset(vaug[:, :, :, DH:DH + 1], 1.0)

            # AT packed by head pairs: [128, dc, s]
            AT = atp.tile([P, 2, S], BF16)

            def emit_qkv_proj(sb):
                for dc in range(2):
                    for w_s, b_row, dst in ((wq_s, bq_row, QT),
                                            (wk_s, bk_row, KT)):
                        ps = psA.tile([P, 512], F32, tag="psA")
                        # K=1 ones matmul seeds the psum with the bias
                        nc.tensor.matmul(
                            ps, b_row[:, dc * P:(dc + 1) * P],
                            ones_s[0:1, :], start=True, stop=False)
                        for c in range(N_CT):
                            nc.tensor.matmul(
                                ps,
                                w_s[:, c, dc * P:(dc + 1) * P],
                                xT[c][:, sb * 512:(sb + 1) * 512],
                                start=False,
                                stop=(c == N_CT - 1),
                            )
                        nc.scalar.activation(
                            dst[:, dc, sb * 512:(sb + 1) * 512], ps, AF.Copy)
                for tt in range(4 * sb, 4 * sb + 4):
                    ps = psA.tile([P, 512], F32, tag="psA")
                    pvs = ps[:, 0:DPC]
                    # K=1 ones matmul seeds the psum with bv broadcast
                    nc.tensor.matmul(ps[:, 0:DPC], ones_s[0:1, 0:P], bv_row,
                                     start=True, stop=False)
                    for c in range(N_CT):
                        nc.tensor.matmul(
                            pvs,
                            xT[c][:, tt * P:(tt + 1) * P],
                            wv_s[:, c, :],
                            start=False,
                            stop=(c == N_CT - 1),
                        )
                    nc.vector.tensor_copy(
                        vaug[:, tt, :, 0:DH],
                        pvs.rearrange("p (h u) -> p h u", h=NH))

            def emit_attention(sb):
                # Head pairs (2*dc, 2*dc+1) share each score/exp tile: the two
                # K=64 score matmuls go to PE row-groups 0 and 64 (concurrent).
                t_cnt = 4 * sb + 4
                for dc in range(2):
                    pvt = [ppv.tile([P, 512], F32, tag="pv",
                                    name=f"pv{sb}_{dc}_{e}")
                           for e in range(2)]
                    # both heads: rows 0..63 = values, row 64 = denominator
                    pv_dst = (pvt[0][0:DH + 1, :], pvt[1][0:DH + 1, :])
                    pend = []  # deferred PV emission: (T, ms, ex)
                    for T in range(t_cnt):
                        k = T - 4 * sb
                        ms = 128 * k if k > 0 else 0
                        sc = psc.tile([P, 2, 512], F32, tag="sc")
                        ex = exw.tile([P, 2, 512], BF16, tag="ex")
                        for e in range(2):  # even/odd head of the pair
                            off = DH * e
                            nc.tensor.matmul(
                                sc[:, e, ms:512],
                                KT[off:off + DH, dc, T * P:(T + 1) * P],
                                QT[off:off + DH, dc,
                                   sb * 512 + ms:(sb + 1) * 512],
                                start=True,
                                stop=True,
                            )
                        nc.scalar.activation(
                            ex[:, :, ms:512], sc[:, :, ms:512],
                            AF.Exp, scale=SCALE,
                        )
                        if k >= 0:  # triangular mask on diagonal chunks
                            nc.gpsimd.affine_select(
                                out=ex[:, :, ms:ms + P],
                                in_=ex[:, :, ms:ms + P],
                                compare_op=ALU.is_ge,
                                fill=0.0,
                                base=0,
                                pattern=[[0, 2], [1, P]],
                                channel_multiplier=-1,
                            )
                        pend.append((T, ms, ex))
                        if len(pend) > 1:
                            _emit_pv(dc, pv_dst, pend.pop(0), t_cnt)
                    _emit_pv(dc, pv_dst, pend.pop(0), t_cnt)

                    if skip_norm:
                        nc.vector.tensor_copy(
                            AT[0:DH, dc, sb * 512:(sb + 1) * 512],
                            pvt[0][0:DH, :])
                        continue
                    # normalize: ACT-copy denom rows to bf16, PE ones-matmul
                    # broadcasts them to 64 psum partitions, one fast-recip
                    # over both heads, DVE multiplies into AT (e=1 via DMA)
                    drow = rcpw.tile([P, 2, 512], BF16, tag="rcp")
                    bcp = psc.tile([P, 2, 512], F32, tag="sc")
                    for e in range(2):
                        nc.scalar.activation(
                            drow[DH:DH + 1, e, :], pvt[e][DH:DH + 1, :],
                            AF.Copy)
                        nc.tensor.matmul(
                            bcp[0:DH, e, :], ones_s[DH:DH + 1, 0:DH],
                            drow[DH:DH + 1, e, :], start=True, stop=True)
                    rcb = bcw.tile([P, 2, 512], F32, tag="bc")
                    nc.vector.reciprocal_approx_fast(
                        out=rcb[0:DH, :, :], in_=bcp[0:DH, :, :])
                    nc.vector.tensor_mul(
                        AT[0:DH, dc, sb * 512:(sb + 1) * 512],
                        pvt[0][0:DH, :], rcb[0:DH, 0, :])
                    att = exw.tile([DH, 512], BF16, tag="att")
                    nc.vector.tensor_mul(att, pvt[1][0:DH, :], rcb[0:DH, 1, :])
                    nc.gpsimd.dma_start(
                        AT[DH:P, dc, sb * 512:(sb + 1) * 512], att)

            def _emit_pv(dc, pv_dst, item, t_cnt):
                T, ms, ex = item
                for e in range(2):
                    h = 2 * dc + e
                    nc.tensor.matmul(
                        pv_dst[e][:, ms:512],
                        vaug[:, T, h, :],
                        ex[:, e, ms:512],
                        start=(T == 0),
                        stop=(T == t_cnt - 1),
                    )

            def emit_outproj(sb):
                for st in range(4 * sb, 4 * sb + 4):
                    ys = ysp.tile([P, 1024], F32, tag="ys")
                    for eb in range(2):
                        ps = psA.tile([P, 512], F32, tag="psA")
                        for dc in range(2):
                            nc.tensor.matmul(
                                ps,
                                AT[:, dc, st * P:(st + 1) * P],
                                wo_s[:, dc, eb * 512:(eb + 1) * 512],
                                start=(dc == 0),
                                stop=(dc == 1),
                            )
                        if eb == 0:
                            nc.scalar.activation(
                                ys[:, 0:512], ps, AF.Copy)
                        else:
                            nc.vector.tensor_copy(ys[:, 512:1024], ps)
                    nc.gpsimd.dma_start(y_d[st * P:(st + 1) * P, :], ys)

            for sb in range(N_SB):
                emit_qkv_proj(sb)
                if sb >= 1:
                    emit_outproj(sb - 1)
                if stop_after <= 1 and sb == 0:
                    break
                emit_attention(sb)
            if stop_after > 1:
                emit_outproj(N_SB - 1)

    return nc


_NC_CACHE = {}


def _get_nc(mm_mode="bf16"):
    if mm_mode not in _NC_CACHE:
        _NC_CACHE[mm_mode] = build_nc(mm_mode=mm_mode)
    return _NC_CACHE[mm_mode]


MM_MODE = "bf16"


def make_in_maps(x, Wq, bq, Wk, bk, Wv, bv, Wo, mm_mode=None):
    """Per-core input dicts: core i -> (batch i//4, head-group i%4)."""
    bf = ml_dtypes.bfloat16
    in_maps = []
    for core in range(8):
        b, g = core // 4, core % 4
        sl = slice(g * DPC, (g + 1) * DPC)
        wqkv = np.concatenate(
            [Wq[:, sl], Wk[:, sl], Wv[:, sl]], axis=0)
        b3 = np.concatenate([bq[sl], bk[sl], bv[sl]])
        in_maps.append({
            "x": np.ascontiguousarray(x[b]).astype(bf),
            "wqkv": np.ascontiguousarray(wqkv).astype(bf),
            "wo": np.ascontiguousarray(Wo[sl, :]).astype(bf),
            "b3": np.ascontiguousarray(b3).astype(np.float32),
        })
    return in_maps


def combine_results(results, bo):
    out = np.zeros((2, S, D), dtype=np.float32)
    for core in range(8):
        out[core // 4] += results[core]["y"]
    out += bo.astype(np.float32)
    return out


_RUNNER_CACHE = {}


def get_runner(mm_mode=None):
    """Build (once) a jitted 8-core runner; returns fn(in_maps) -> results."""
    mode = mm_mode or MM_MODE
    if mode in _RUNNER_CACHE:
        return _RUNNER_CACHE[mode]

    import jax
    from jax.sharding import Mesh, PartitionSpec
    from jax.experimental.shard_map import shard_map
    from concourse import bass2jax, mybir as _mb

    nc = _get_nc(mode)
    bass2jax.install_neuronx_cc_hook()

    pname = nc.partition_id_tensor.name if nc.partition_id_tensor else None
    in_names, out_names, out_avals = [], [], []
    for alloc in nc.m.functions[0].allocations:
        if not isinstance(alloc, _mb.MemoryLocationSet):
            continue
        name = alloc.memorylocations[0].name
        if alloc.kind == "ExternalInput":
            if name != pname:
                in_names.append(name)
        elif alloc.kind == "ExternalOutput":
            out_names.append(name)
            out_avals.append(jax.core.ShapedArray(
                tuple(alloc.tensor_shape), _mb.dt.np(alloc.dtype)))
    n_params = len(in_names)
    all_names = in_names + out_names
    if pname is not None:
        all_names = all_names + [pname]

    def _body(*args):
        operands = list(args)
        if pname is not None:
            operands.append(bass2jax.partition_id_tensor())
        outs = bass2jax._bass_exec_p.bind(
            *operands,
            out_avals=tuple(out_avals),
            in_names=tuple(all_names),
            out_names=tuple(out_names),
            lowering_input_output_aliases=(),
            sim_require_finite=True,
            sim_require_nnan=True,
            nc=nc,
        )
        return tuple(outs)

    devices = jax.devices()[:8]
    mesh = Mesh(np.asarray(devices), ("core",))
    sharded = jax.jit(
        shard_map(_body, mesh=mesh,
                  in_specs=(PartitionSpec("core"),) * (n_params + len(out_names)),
                  out_specs=(PartitionSpec("core"),) * len(out_names),
                  check_rep=False),
        keep_unused=True,
    )

    from jax.sharding import NamedSharding
    zero_outs = [
        jax.device_put(
            np.zeros((8 * a.shape[0], *a.shape[1:]), a.dtype),
            NamedSharding(mesh, PartitionSpec("core")),
        )
        for a in out_avals
    ]

    def run(in_maps):
        concat_in = [
            np.concatenate([np.asarray(m[name]) for m in in_maps], axis=0)
            for name in in_names
        ]
        out_arrs = sharded(*concat_in, *zero_outs)
        return [
            {name: np.asarray(out_arrs[i]).reshape(8, *out_avals[i].shape)[c]
             for i, name in enumerate(out_names)}
            for c in range(8)
        ]

    run.sharded = sharded
    run.in_names = in_names
    run.out_names = out_names
    run.out_avals = out_avals
    run.zero_outs = zero_outs
    _RUNNER_CACHE[mode] = run
    return run


def kernel(x, Wq, bq, Wk, bk, Wv, bv, Wo, bo, **_ignored):
    x = np.asarray(x, dtype=np.float32)
    in_maps = make_in_maps(
        x,
        np.asarray(Wq, np.float32), np.asarray(bq, np.float32),
        np.asarray(Wk, np.float32), np.asarray(bk, np.float32),
        np.asarray(Wv, np.float32), np.asarray(bv, np.float32),
        np.asarray(Wo, np.float32),
    )
    try:
        results = get_runner(MM_MODE)(in_maps)
    except Exception:
        # fallback: stock SPMD runner (slower dispatch, same NEFF)
        from concourse.bass_utils import run_bass_kernel_spmd
        results = run_bass_kernel_spmd(
            _get_nc(MM_MODE), in_maps, core_ids=list(range(8))).results
    return combine_results(results, np.asarray(bo, np.float32))


# revision 25
# speedup vs baseline: 402.7732x; 1.0554x over previous
"""Causal self-attention (D=1024, H=16, S=2048, B=2) on 8 trn2 cores.

Sharding: core i handles batch b = i // 4 and head-group g = i % 4
(4 heads = 256 model dims per group). Each core computes
    y_partial[b,g] = softmax_causal(Q K^T / 8) V  @ Wo[rows of g]
for its 4 heads; the host sums the 4 group partials per batch and adds bo.

Per-core kernel (bf16 matmul operands, fp32 PSUM accumulation), emitted
as a software pipeline over 512-column s-blocks:
  per sb: QT/KT = (Wq/Wk)^T x^T + b (ACT Identity-with-bias psum moves),
          V tiles (DVE bias adds), then attention for both head pairs:
          scoresT = KT^T QT on PE row-group pairs, exp on ACT, triangular
          diag masks on Pool, PV accumulation with the softmax-denominator
          ones-column trick. Head e=0 packs V|1 (denom at psum row 64),
          head e=1 packs 1|V and lands at psum partitions 63..127 so both
          heads normalize lane-aligned: DVE fast-reciprocal of the denom
          row, Pool partition_broadcast, DVE multiply -> AT bf16.
          Output projection y = A Wo trails one s-block behind.
DMA queues: x DMA-transposes split across sync+tensor queues; weights on
gpsimd; y stores on gpsimd/sync (cheap DGE issue paths).
"""

import sys

sys.path.insert(0, "/opt/trn_rl_repo")

import ml_dtypes
import numpy as np

import concourse.bass as bass
import concourse.mybir as mybir
import concourse.tile as tile
from concourse import bacc

P = 128
S = 2048
D = 1024
NH = 4                    # heads per core
DH = 64                   # head dim
DPC = NH * DH             # model dims per core = 256
N_CT = D // P             # 8 contraction chunks
N_ST = S // P             # 16 t tiles of 128
N_SB = S // 512           # 4 s blocks of 512
F32 = mybir.dt.float32
BF16 = mybir.dt.bfloat16
SCALE = 1.0 / 8.0         # 1/sqrt(64)

AF = mybir.ActivationFunctionType
ALU = mybir.AluOpType


def build_nc(mm_mode: str = "bf16", stop_after: int = 99,
             skip_norm: bool = False) -> bass.Bass:
    nc = _build(mm_mode, stop_after, skip_norm)
    if not nc.is_finalized():
        nc.finalize()
    return nc


def _build(mm_mode: str, stop_after: int, skip_norm: bool) -> bass.Bass:
    assert mm_mode == "bf16"
    nc = bacc.Bacc("TRN2", target_bir_lowering=False, debug=False,
                   num_devices=8)

    x_d = nc.dram_tensor("x", [S, D], BF16, kind="ExternalInput")
    # wq/wk/wv stacked host-side: one DMA instead of three
    wqkv_d = nc.dram_tensor("wqkv", [3 * D, DPC], BF16, kind="ExternalInput")
    wo_d = nc.dram_tensor("wo", [DPC, D], BF16, kind="ExternalInput")
    # bq|bk|bv stacked host-side: one DMA instead of three
    b3_d = nc.dram_tensor("b3", [3 * DPC], F32, kind="ExternalInput")
    y_d = nc.dram_tensor("y", [S, D], F32, kind="ExternalOutput")

    with tile.TileContext(nc) as tc:
        with (
            tc.tile_pool(name="const", bufs=1) as const,
            tc.tile_pool(name="xtp", bufs=1) as xtp,
            tc.tile_pool(name="qkv", bufs=1) as qkv,
            tc.tile_pool(name="atp", bufs=1) as atp,
            tc.tile_pool(name="exw", bufs=3) as exw,
            tc.tile_pool(name="rcpw", bufs=2) as rcpw,
            tc.tile_pool(name="bcw", bufs=2) as bcw,
            tc.tile_pool(name="ysp", bufs=4) as ysp,
            tc.tile_pool(name="psA", bufs=2, space="PSUM") as psA,
            tc.tile_pool(name="psc", bufs=2, space="PSUM") as psc,
            tc.tile_pool(name="ppv", bufs=2, space="PSUM") as ppv,
        ):
            # ---- weights / constants: 3 DMAs total on the sync HWDGE
            # queue, ahead of the x transposes (the DMA semaphore ring
            # serializes across queues, so every early DMA counts) ----
            wqkv_s = const.tile([P, 3 * N_CT, DPC], BF16)
            nc.sync.dma_start(wqkv_s,
                              wqkv_d.rearrange("(o p) d -> p o d", p=P))
            wq_s = wqkv_s[:, 0 * N_CT:1 * N_CT, :]
            wk_s = wqkv_s[:, 1 * N_CT:2 * N_CT, :]
            wv_s = wqkv_s[:, 2 * N_CT:3 * N_CT, :]
            # Wo packed by head pairs: rows 128*dc .. 128*dc+127
            wo_s = const.tile([P, 2, D], BF16)
            nc.sync.dma_start(wo_s, wo_d.rearrange("(dc p) e -> p dc e", p=P))

            # biases as bf16 rows on partition 0; applied by K=1
            # ones-stationary matmuls that seed the projection psums
            b3_f32 = const.tile([1, 3 * DPC], F32)
            nc.sync.dma_start(b3_f32, b3_d[:].unsqueeze(0))
            b3_row = const.tile([1, 3 * DPC], BF16)
            nc.vector.tensor_copy(b3_row, b3_f32)
            bq_row = b3_row[:, 0 * DPC:1 * DPC]
            bk_row = b3_row[:, 1 * DPC:2 * DPC]
            bv_row = b3_row[:, 2 * DPC:3 * DPC]
            # bf16 ones: K=1 stationary / moving rows for broadcasts
            ones_s = const.tile([P, 512], BF16)
            nc.vector.memset(ones_s, 1.0)
            # bf16 identity for PE-transposes of x
            ident = const.tile([P, P], BF16)
            nc.gpsimd.affine_select(
                out=ident, in_=ones_s[:, 0:P], compare_op=ALU.is_equal,
                fill=0.0, base=0, pattern=[[1, P]], channel_multiplier=-1)

            # ---- phase 0: x loaded straight (2 DMAs), transposed on the
            # otherwise-idle PE via identity matmuls. The DMA semaphore
            # ring serializes all queues, so replacing 16 XBAR transposes
            # with 2 plain loads removes ~30us of startup ladder.
            xT = [xtp.tile([P, S], BF16, tag=f"xt{c}", name=f"xt{c}")
                  for c in range(N_CT)]
            x_nat = [xtp.tile([P, 8, 1024], BF16, tag=f"xn{h}",
                              name=f"xn{h}") for h in range(2)]
            for h in range(2):
                nc.sync.dma_start(
                    x_nat[h],
                    x_d[h * 1024:(h + 1) * 1024, :]
                    .rearrange("(tt p) d -> p tt d", p=P))

            def emit_xpose(g):
                h, q = divmod(g, 2)
                for cp in range(4):  # chunk pairs
                    tp = psA.tile([P, 1024], BF16, tag="psA",
                                  name=f"tp{g}_{cp}")
                    for ci in range(2):
                        c = 2 * cp + ci
                        for B in range(4):
                            tt = q * 4 + B
                            nc.tensor.matmul(
                                tp[:, ci * 512 + B * P:
                                   ci * 512 + (B + 1) * P],
                                x_nat[h][:, tt, c * P:(c + 1) * P],
                                ident, is_transpose=True,
                                start=True, stop=True)
                    for ci in range(2):
                        c = 2 * cp + ci
                        sl = tp[:, ci * 512:(ci + 1) * 512]
                        dst = xT[c][:, g * 512:(g + 1) * 512]
                        if cp % 2 == 0:
                            nc.scalar.activation(dst, sl, AF.Copy)
                        else:
                            nc.vector.tensor_copy(dst, sl)

            # QT/KT: [128 (head-pair d), dc, s]
            QT = qkv.tile([P, 2, S], BF16)
            KT = qkv.tile([P, 2, S], BF16)
            # V_aug: [t-part, t-chunk, head, 65], col 64 == 1.0 so the PV
            # matmul's psum row 64 accumulates the softmax denominator.
            vaug = qkv.tile([P, N_ST, NH, DH + 1], BF16)
            nc.vector.memset(vaug[:, :, :, DH:DH + 1], 1.0)

            # AT packed by head pairs: [128, dc, s]
            AT = atp.tile([P, 2, S], BF16)

            def emit_qkv_proj(sb):
                for dc in range(2):
                    for w_s, b_row, dst in ((wq_s, bq_row, QT),
                                            (wk_s, bk_row, KT)):
                        ps = psA.tile([P, 512], F32, tag="psA")
                        # K=1 ones matmul seeds the psum with the bias
                        nc.tensor.matmul(
                            ps, b_row[:, dc * P:(dc + 1) * P],
                            ones_s[0:1, :], start=True, stop=False)
                        for c in range(N_CT):
                            nc.tensor.matmul(
                                ps,
                                w_s[:, c, dc * P:(dc + 1) * P],
                                xT[c][:, sb * 512:(sb + 1) * 512],
                                start=False,
                                stop=(c == N_CT - 1),
                            )
                        nc.scalar.activation(
                            dst[:, dc, sb * 512:(sb + 1) * 512], ps, AF.Copy)
                for tt in range(4 * sb, 4 * sb + 4):
                    ps = psA.tile([P, 512], F32, tag="psA")
                    pvs = ps[:, 0:DPC]
                    # K=1 ones matmul seeds the psum with bv broadcast
                    nc.tensor.matmul(ps[:, 0:DPC], ones_s[0:1, 0:P], bv_row,
                                     start=True, stop=False)
                    for c in range(N_CT):
                        nc.tensor.matmul(
                            pvs,
                            xT[c][:, tt * P:(tt + 1) * P],
                            wv_s[:, c, :],
                            start=False,
                            stop=(c == N_CT - 1),
                        )
                    nc.vector.tensor_copy(
                        vaug[:, tt, :, 0:DH],
                        pvs.rearrange("p (h u) -> p h u", h=NH))

            def emit_attention(sb):
                # Head pairs (2*dc, 2*dc+1) share each score/exp tile: the two
                # K=64 score matmuls go to PE row-groups 0 and 64 (concurrent).
                t_cnt = 4 * sb + 4
                for dc in range(2):
                    pvt = [ppv.tile([P, 512], F32, tag="pv",
                                    name=f"pv{sb}_{dc}_{e}")
                           for e in range(2)]
                    # both heads: rows 0..63 = values, row 64 = denominator
                    pv_dst = (pvt[0][0:DH + 1, :], pvt[1][0:DH + 1, :])
                    pend = []  # deferred PV emission: (T, ms, ex)
                    for T in range(t_cnt):
                        k = T - 4 * sb
                        ms = 128 * k if k > 0 else 0
                        sc = psc.tile([P, 2, 512], F32, tag="sc")
                        ex = exw.tile([P, 2, 512], BF16, tag="ex")
                        for e in range(2):  # even/odd head of the pair
                            off = DH * e
                            nc.tensor.matmul(
                                sc[:, e, ms:512],
                                KT[off:off + DH, dc, T * P:(T + 1) * P],
                                QT[off:off + DH, dc,
                                   sb * 512 + ms:(sb + 1) * 512],
                                start=True,
                                stop=True,
                            )
                        nc.scalar.activation(
                            ex[:, :, ms:512], sc[:, :, ms:512],
                            AF.Exp, scale=SCALE,
                        )
                        if k >= 0:  # triangular mask on diagonal chunks
                            nc.gpsimd.affine_select(
                                out=ex[:, :, ms:ms + P],
                                in_=ex[:, :, ms:ms + P],
                                compare_op=ALU.is_ge,
                                fill=0.0,
                                base=0,
                                pattern=[[0, 2], [1, P]],
                                channel_multiplier=-1,
                            )
                        pend.append((T, ms, ex))
                        if len(pend) > 1:
                            _emit_pv(dc, pv_dst, pend.pop(0), t_cnt)
                    _emit_pv(dc, pv_dst, pend.pop(0), t_cnt)

                    if skip_norm:
                        nc.vector.tensor_copy(
                            AT[0:DH, dc, sb * 512:(sb + 1) * 512],
                            pvt[0][0:DH, :])
                        continue
                    # normalize: ACT-copy denom rows to bf16, PE ones-matmul
                    # broadcasts them to 64 psum partitions, one fast-recip
                    # over both heads, DVE multiplies into AT (e=1 via DMA)
                    drow = rcpw.tile([P, 2, 512], BF16, tag="rcp")
                    bcp = psc.tile([P, 2, 512], F32, tag="sc")
                    for e in range(2):
                        nc.scalar.activation(
                            drow[DH:DH + 1, e, :], pvt[e][DH:DH + 1, :],
                            AF.Copy)
                        nc.tensor.matmul(
                            bcp[0:DH, e, :], ones_s[DH:DH + 1, 0:DH],
                            drow[DH:DH + 1, e, :], start=True, stop=True)
                    rcb = bcw.tile([P, 2, 512], F32, tag="bc")
                    nc.vector.reciprocal_approx_fast(
                        out=rcb[0:DH, :, :], in_=bcp[0:DH, :, :])
                    nc.vector.tensor_mul(
                        AT[0:DH, dc, sb * 512:(sb + 1) * 512],
                        pvt[0][0:DH, :], rcb[0:DH, 0, :])
                    att = exw.tile([DH, 512], BF16, tag="att")
                    nc.vector.tensor_mul(att, pvt[1][0:DH, :], rcb[0:DH, 1, :])
                    nc.gpsimd.dma_start(
                        AT[DH:P, dc, sb * 512:(sb + 1) * 512], att)

            def _emit_pv(dc, pv_dst, item, t_cnt):
                T, ms, ex = item
                for e in range(2):
                    h = 2 * dc + e
                    nc.tensor.matmul(
                        pv_dst[e][:, ms:512],
                        vaug[:, T, h, :],
                        ex[:, e, ms:512],
                        start=(T == 0),
                        stop=(T == t_cnt - 1),
                    )

            def emit_outproj(sb):
                for st in range(4 * sb, 4 * sb + 4):
                    ys = ysp.tile([P, 1024], F32, tag="ys")
                    for eb in range(2):
                        ps = psA.tile([P, 512], F32, tag="psA")
                        for dc in range(2):
                            nc.tensor.matmul(
                                ps,
                                AT[:, dc, st * P:(st + 1) * P],
                                wo_s[:, dc, eb * 512:(eb + 1) * 512],
                                start=(dc == 0),
                                stop=(dc == 1),
                            )
                        if eb == 0:
                            nc.scalar.activation(
                                ys[:, 0:512], ps, AF.Copy)
                        else:
                            nc.vector.tensor_copy(ys[:, 512:1024], ps)
                    nc.sync.dma_start(y_d[st * P:(st + 1) * P, :], ys)

            emit_xpose(0)
            for sb in range(N_SB):
                emit_qkv_proj(sb)
                if sb + 1 < N_SB:
                    emit_xpose(sb + 1)
                if sb >= 1:
                    emit_outproj(sb - 1)
                if stop_after <= 1 and sb == 0:
                    break
                emit_attention(sb)
            if stop_after > 1:
                emit_outproj(N_SB - 1)

    return nc


_NC_CACHE = {}


def _get_nc(mm_mode="bf16"):
    if mm_mode not in _NC_CACHE:
        _NC_CACHE[mm_mode] = build_nc(mm_mode=mm_mode)
    return _NC_CACHE[mm_mode]


MM_MODE = "bf16"


def make_in_maps(x, Wq, bq, Wk, bk, Wv, bv, Wo, mm_mode=None):
    """Per-core input dicts: core i -> (batch i//4, head-group i%4)."""
    bf = ml_dtypes.bfloat16
    in_maps = []
    for core in range(8):
        b, g = core // 4, core % 4
        sl = slice(g * DPC, (g + 1) * DPC)
        wqkv = np.concatenate(
            [Wq[:, sl], Wk[:, sl], Wv[:, sl]], axis=0)
        b3 = np.concatenate([bq[sl], bk[sl], bv[sl]])
        in_maps.append({
            "x": np.ascontiguousarray(x[b]).astype(bf),
            "wqkv": np.ascontiguousarray(wqkv).astype(bf),
            "wo": np.ascontiguousarray(Wo[sl, :]).astype(bf),
            "b3": np.ascontiguousarray(b3).astype(np.float32),
        })
    return in_maps


def combine_results(results, bo):
    out = np.zeros((2, S, D), dtype=np.float32)
    for core in range(8):
        out[core // 4] += results[core]["y"]
    out += bo.astype(np.float32)
    return out


_RUNNER_CACHE = {}


def get_runner(mm_mode=None):
    """Build (once) a jitted 8-core runner; returns fn(in_maps) -> results."""
    mode = mm_mode or MM_MODE
    if mode in _RUNNER_CACHE:
        return _RUNNER_CACHE[mode]

    import jax
    from jax.sharding import Mesh, PartitionSpec
    from jax.experimental.shard_map import shard_map
    from concourse import bass2jax, mybir as _mb

    nc = _get_nc(mode)
    bass2jax.install_neuronx_cc_hook()

    pname = nc.partition_id_tensor.name if nc.partition_id_tensor else None
    in_names, out_names, out_avals = [], [], []
    for alloc in nc.m.functions[0].allocations:
        if not isinstance(alloc, _mb.MemoryLocationSet):
            continue
        name = alloc.memorylocations[0].name
        if alloc.kind == "ExternalInput":
            if name != pname:
                in_names.append(name)
        elif alloc.kind == "ExternalOutput":
            out_names.append(name)
            out_avals.append(jax.core.ShapedArray(
                tuple(alloc.tensor_shape), _mb.dt.np(alloc.dtype)))
    n_params = len(in_names)
    all_names = in_names + out_names
    if pname is not None:
        all_names = all_names + [pname]

    def _body(*args):
        operands = list(args)
        if pname is not None:
            operands.append(bass2jax.partition_id_tensor())
        outs = bass2jax._bass_exec_p.bind(
            *operands,
            out_avals=tuple(out_avals),
            in_names=tuple(all_names),
            out_names=tuple(out_names),
            lowering_input_output_aliases=(),
            sim_require_finite=True,
            sim_require_nnan=True,
            nc=nc,
        )
        return tuple(outs)

    devices = jax.devices()[:8]
    mesh = Mesh(np.asarray(devices), ("core",))
    sharded = jax.jit(
        shard_map(_body, mesh=mesh,
                  in_specs=(PartitionSpec("core"),) * (n_params + len(out_names)),
                  out_specs=(PartitionSpec("core"),) * len(out_names),
                  check_rep=False),
        keep_unused=True,
    )

    from jax.sharding import NamedSharding
    zero_outs = [
        jax.device_put(
            np.zeros((8 * a.shape[0], *a.shape[1:]), a.dtype),
            NamedSharding(mesh, PartitionSpec("core")),
        )
        for a in out_avals
    ]

    def run(in_maps):
        concat_in = [
            np.concatenate([np.asarray(m[name]) for m in in_maps], axis=0)
            for name in in_names
        ]
        out_arrs = sharded(*concat_in, *zero_outs)
        return [
            {name: np.asarray(out_arrs[i]).reshape(8, *out_avals[i].shape)[c]
             for i, name in enumerate(out_names)}
            for c in range(8)
        ]

    run.sharded = sharded
    run.in_names = in_names
    run.out_names = out_names
    run.out_avals = out_avals
    run.zero_outs = zero_outs
    _RUNNER_CACHE[mode] = run
    return run


def kernel(x, Wq, bq, Wk, bk, Wv, bv, Wo, bo, **_ignored):
    x = np.asarray(x, dtype=np.float32)
    in_maps = make_in_maps(
        x,
        np.asarray(Wq, np.float32), np.asarray(bq, np.float32),
        np.asarray(Wk, np.float32), np.asarray(bk, np.float32),
        np.asarray(Wv, np.float32), np.asarray(bv, np.float32),
        np.asarray(Wo, np.float32),
    )
    try:
        results = get_runner(MM_MODE)(in_maps)
    except Exception:
        # fallback: stock SPMD runner (slower dispatch, same NEFF)
        from concourse.bass_utils import run_bass_kernel_spmd
        results = run_bass_kernel_spmd(
            _get_nc(MM_MODE), in_maps, core_ids=list(range(8))).results
    return combine_results(results, np.asarray(bo, np.float32))


# revision 30
# speedup vs baseline: 402.8745x; 1.0003x over previous
"""Causal self-attention (D=1024, H=16, S=2048, B=2) on 8 trn2 cores.

Sharding: core i handles batch b = i // 4 and head-group g = i % 4
(4 heads = 256 model dims per group). Each core computes
    y_partial[b,g] = softmax_causal(Q K^T / 8) V  @ Wo[rows of g]
for its 4 heads; the host sums the 4 group partials per batch and adds bo.

Per-core kernel (bf16 matmul operands, fp32 PSUM accumulation), emitted
as a software pipeline over 512-column s-blocks:
  per sb: QT/KT = (Wq/Wk)^T x^T + b (ACT Identity-with-bias psum moves),
          V tiles (DVE bias adds), then attention for both head pairs:
          scoresT = KT^T QT on PE row-group pairs, exp on ACT, triangular
          diag masks on Pool, PV accumulation with the softmax-denominator
          ones-column trick. Head e=0 packs V|1 (denom at psum row 64),
          head e=1 packs 1|V and lands at psum partitions 63..127 so both
          heads normalize lane-aligned: DVE fast-reciprocal of the denom
          row, Pool partition_broadcast, DVE multiply -> AT bf16.
          Output projection y = A Wo trails one s-block behind.
DMA queues: x DMA-transposes split across sync+tensor queues; weights on
gpsimd; y stores on gpsimd/sync (cheap DGE issue paths).
"""

import sys

sys.path.insert(0, "/opt/trn_rl_repo")

import ml_dtypes
import numpy as np

import concourse.bass as bass
import concourse.mybir as mybir
import concourse.tile as tile
from concourse import bacc

P = 128
S = 2048
D = 1024
NH = 4                    # heads per core
DH = 64                   # head dim
DPC = NH * DH             # model dims per core = 256
N_CT = D // P             # 8 contraction chunks
N_ST = S // P             # 16 t tiles of 128
N_SB = S // 512           # 4 s blocks of 512
F32 = mybir.dt.float32
BF16 = mybir.dt.bfloat16
SCALE = 1.0 / 8.0         # 1/sqrt(64)

AF = mybir.ActivationFunctionType
ALU = mybir.AluOpType


def build_nc(mm_mode: str = "bf16", stop_after: int = 99,
             skip_norm: bool = False) -> bass.Bass:
    nc = _build(mm_mode, stop_after, skip_norm)
    if not nc.is_finalized():
        nc.finalize()
    return nc


def _build(mm_mode: str, stop_after: int, skip_norm: bool) -> bass.Bass:
    assert mm_mode == "bf16"
    nc = bacc.Bacc("TRN2", target_bir_lowering=False, debug=False,
                   num_devices=8)

    x_d = nc.dram_tensor("x", [S, D], BF16, kind="ExternalInput")
    # wq/wk/wv stacked host-side: one DMA instead of three
    wqkv_d = nc.dram_tensor("wqkv", [3 * D, DPC], BF16, kind="ExternalInput")
    wo_d = nc.dram_tensor("wo", [DPC, D], BF16, kind="ExternalInput")
    # bq|bk|bv stacked host-side: one DMA instead of three
    b3_d = nc.dram_tensor("b3", [3 * DPC], F32, kind="ExternalInput")
    # bf16 partials: host sums 4 of them per batch in fp32
    y_d = nc.dram_tensor("y", [S, D], BF16, kind="ExternalOutput")

    with tile.TileContext(nc) as tc:
        with (
            tc.tile_pool(name="const", bufs=1) as const,
            tc.tile_pool(name="xtp", bufs=1) as xtp,
            tc.tile_pool(name="qkv", bufs=1) as qkv,
            tc.tile_pool(name="atp", bufs=1) as atp,
            tc.tile_pool(name="exw", bufs=3) as exw,
            tc.tile_pool(name="rcpw", bufs=2) as rcpw,
            tc.tile_pool(name="bcw", bufs=2) as bcw,
            tc.tile_pool(name="ysp", bufs=4) as ysp,
            tc.tile_pool(name="psA", bufs=2, space="PSUM") as psA,
            tc.tile_pool(name="psc", bufs=2, space="PSUM") as psc,
            tc.tile_pool(name="ppv", bufs=2, space="PSUM") as ppv,
        ):
            # ---- weights / constants: 3 DMAs total on the sync HWDGE
            # queue, ahead of the x transposes (the DMA semaphore ring
            # serializes across queues, so every early DMA counts) ----
            # x's first half leads the queue so PE transposes start ASAP
            x_nat = [xtp.tile([P, 8, 1024], BF16, tag=f"xn{h}",
                              name=f"xn{h}") for h in range(2)]
            nc.sync.dma_start(
                x_nat[0],
                x_d[0:1024, :].rearrange("(tt p) d -> p tt d", p=P))
            wqkv_s = const.tile([P, 3 * N_CT, DPC], BF16)
            nc.sync.dma_start(wqkv_s,
                              wqkv_d.rearrange("(o p) d -> p o d", p=P))
            wq_s = wqkv_s[:, 0 * N_CT:1 * N_CT, :]
            wk_s = wqkv_s[:, 1 * N_CT:2 * N_CT, :]
            wv_s = wqkv_s[:, 2 * N_CT:3 * N_CT, :]
            # Wo packed by head pairs: rows 128*dc .. 128*dc+127
            wo_s = const.tile([P, 2, D], BF16)
            nc.sync.dma_start(wo_s, wo_d.rearrange("(dc p) e -> p dc e", p=P))

            # biases as bf16 rows on partition 0; applied by K=1
            # ones-stationary matmuls that seed the projection psums
            b3_f32 = const.tile([1, 3 * DPC], F32)
            nc.sync.dma_start(b3_f32, b3_d[:].unsqueeze(0))
            b3_row = const.tile([1, 3 * DPC], BF16)
            nc.vector.tensor_copy(b3_row, b3_f32)
            bq_row = b3_row[:, 0 * DPC:1 * DPC]
            bk_row = b3_row[:, 1 * DPC:2 * DPC]
            bv_row = b3_row[:, 2 * DPC:3 * DPC]
            # bf16 ones: K=1 stationary / moving rows for broadcasts
            ones_s = const.tile([P, 512], BF16)
            nc.vector.memset(ones_s, 1.0)
            # bf16 identity for PE-transposes of x
            ident = const.tile([P, P], BF16)
            nc.gpsimd.affine_select(
                out=ident, in_=ones_s[:, 0:P], compare_op=ALU.is_equal,
                fill=0.0, base=0, pattern=[[1, P]], channel_multiplier=-1)

            # ---- phase 0: x loaded straight (2 DMAs), transposed on the
            # otherwise-idle PE via identity matmuls. The DMA semaphore
            # ring serializes all queues, so replacing 16 XBAR transposes
            # with 2 plain loads removes ~30us of startup ladder.
            xT = [xtp.tile([P, S], BF16, tag=f"xt{c}", name=f"xt{c}")
                  for c in range(N_CT)]
            nc.sync.dma_start(
                x_nat[1],
                x_d[1024:2048, :].rearrange("(tt p) d -> p tt d", p=P))

            def emit_xpose(g):
                h, q = divmod(g, 2)
                for cp in range(4):  # chunk pairs
                    tp = psA.tile([P, 1024], BF16, tag="psA",
                                  name=f"tp{g}_{cp}")
                    for ci in range(2):
                        c = 2 * cp + ci
                        for B in range(4):
                            tt = q * 4 + B
                            nc.tensor.matmul(
                                tp[:, ci * 512 + B * P:
                                   ci * 512 + (B + 1) * P],
                                x_nat[h][:, tt, c * P:(c + 1) * P],
                                ident, is_transpose=True,
                                start=True, stop=True)
                    for ci in range(2):
                        c = 2 * cp + ci
                        sl = tp[:, ci * 512:(ci + 1) * 512]
                        dst = xT[c][:, g * 512:(g + 1) * 512]
                        if cp % 2 == 0:
                            nc.scalar.activation(dst, sl, AF.Copy)
                        else:
                            nc.vector.tensor_copy(dst, sl)

            # QT/KT: [128 (head-pair d), dc, s]
            QT = qkv.tile([P, 2, S], BF16)
            KT = qkv.tile([P, 2, S], BF16)
            # V_aug: [t-part, t-chunk, head, 65], col 64 == 1.0 so the PV
            # matmul's psum row 64 accumulates the softmax denominator.
            vaug = qkv.tile([P, N_ST, NH, DH + 1], BF16)
            nc.vector.memset(vaug[:, :, :, DH:DH + 1], 1.0)

            # AT packed by head pairs: [128, dc, s]
            AT = atp.tile([P, 2, S], BF16)

            def emit_qkv_proj(sb):
                for dc in range(2):
                    for w_s, b_row, dst in ((wq_s, bq_row, QT),
                                            (wk_s, bk_row, KT)):
                        ps = psA.tile([P, 512], F32, tag="psA")
                        # K=1 ones matmul seeds the psum with the bias
                        nc.tensor.matmul(
                            ps, b_row[:, dc * P:(dc + 1) * P],
                            ones_s[0:1, :], start=True, stop=False)
                        for c in range(N_CT):
                            nc.tensor.matmul(
                                ps,
                                w_s[:, c, dc * P:(dc + 1) * P],
                                xT[c][:, sb * 512:(sb + 1) * 512],
                                start=False,
                                stop=(c == N_CT - 1),
                            )
                        nc.scalar.activation(
                            dst[:, dc, sb * 512:(sb + 1) * 512], ps, AF.Copy)
                for tt in range(4 * sb, 4 * sb + 4):
                    ps = psA.tile([P, 512], F32, tag="psA")
                    pvs = ps[:, 0:DPC]
                    # K=1 ones matmul seeds the psum with bv broadcast
                    nc.tensor.matmul(ps[:, 0:DPC], ones_s[0:1, 0:P], bv_row,
                                     start=True, stop=False)
                    for c in range(N_CT):
                        nc.tensor.matmul(
                            pvs,
                            xT[c][:, tt * P:(tt + 1) * P],
                            wv_s[:, c, :],
                            start=False,
                            stop=(c == N_CT - 1),
                        )
                    nc.vector.tensor_copy(
                        vaug[:, tt, :, 0:DH],
                        pvs.rearrange("p (h u) -> p h u", h=NH))

            def emit_attention(sb):
                # Head pairs (2*dc, 2*dc+1) share each score/exp tile: the two
                # K=64 score matmuls go to PE row-groups 0 and 64 (concurrent).
                t_cnt = 4 * sb + 4
                for dc in range(2):
                    pvt = [ppv.tile([P, 512], F32, tag="pv",
                                    name=f"pv{sb}_{dc}_{e}")
                           for e in range(2)]
                    # both heads: rows 0..63 = values, row 64 = denominator
                    pv_dst = (pvt[0][0:DH + 1, :], pvt[1][0:DH + 1, :])
                    pend = []  # deferred PV emission: (T, ms, ex)
                    for T in range(t_cnt):
                        k = T - 4 * sb
                        ms = 128 * k if k > 0 else 0
                        sc = psc.tile([P, 2, 512], F32, tag="sc")
                        ex = exw.tile([P, 2, 512], BF16, tag="ex")
                        for e in range(2):  # even/odd head of the pair
                            off = DH * e
                            nc.tensor.matmul(
                                sc[:, e, ms:512],
                                KT[off:off + DH, dc, T * P:(T + 1) * P],
                                QT[off:off + DH, dc,
                                   sb * 512 + ms:(sb + 1) * 512],
                                start=True,
                                stop=True,
                            )
                        nc.scalar.activation(
                            ex[:, :, ms:512], sc[:, :, ms:512],
                            AF.Exp, scale=SCALE,
                        )
                        if k >= 0:  # triangular mask on diagonal chunks
                            nc.gpsimd.affine_select(
                                out=ex[:, :, ms:ms + P],
                                in_=ex[:, :, ms:ms + P],
                                compare_op=ALU.is_ge,
                                fill=0.0,
                                base=0,
                                pattern=[[0, 2], [1, P]],
                                channel_multiplier=-1,
                            )
                        pend.append((T, ms, ex))
                        if len(pend) > 1:
                            _emit_pv(dc, pv_dst, pend.pop(0), t_cnt)
                    _emit_pv(dc, pv_dst, pend.pop(0), t_cnt)

                    if skip_norm:
                        nc.vector.tensor_copy(
                            AT[0:DH, dc, sb * 512:(sb + 1) * 512],
                            pvt[0][0:DH, :])
                        continue
                    # normalize: ACT-copy denom rows to bf16, PE ones-matmul
                    # broadcasts them to 64 psum partitions, one fast-recip
                    # over both heads, DVE multiplies into AT (e=1 via DMA)
                    drow = rcpw.tile([P, 2, 512], BF16, tag="rcp")
                    bcp = psc.tile([P, 2, 512], F32, tag="sc")
                    for e in range(2):
                        nc.scalar.activation(
                            drow[DH:DH + 1, e, :], pvt[e][DH:DH + 1, :],
                            AF.Copy)
                        nc.tensor.matmul(
                            bcp[0:DH, e, :], ones_s[DH:DH + 1, 0:DH],
                            drow[DH:DH + 1, e, :], start=True, stop=True)
                    rcb = bcw.tile([P, 2, 512], F32, tag="bc")
                    nc.vector.reciprocal_approx_fast(
                        out=rcb[0:DH, :, :], in_=bcp[0:DH, :, :])
                    nc.vector.tensor_mul(
                        AT[0:DH, dc, sb * 512:(sb + 1) * 512],
                        pvt[0][0:DH, :], rcb[0:DH, 0, :])
                    att = exw.tile([DH, 512], BF16, tag="att")
                    nc.vector.tensor_mul(att, pvt[1][0:DH, :], rcb[0:DH, 1, :])
                    nc.gpsimd.dma_start(
                        AT[DH:P, dc, sb * 512:(sb + 1) * 512], att)

            def _emit_pv(dc, pv_dst, item, t_cnt):
                T, ms, ex = item
                for e in range(2):
                    h = 2 * dc + e
                    nc.tensor.matmul(
                        pv_dst[e][:, ms:512],
                        vaug[:, T, h, :],
                        ex[:, e, ms:512],
                        start=(T == 0),
                        stop=(T == t_cnt - 1),
                    )

            def emit_outproj(sb):
                for st in range(4 * sb, 4 * sb + 4):
                    ys = ysp.tile([P, 1024], BF16, tag="ys")
                    for eb in range(2):
                        ps = psA.tile([P, 512], F32, tag="psA")
                        for dc in range(2):
                            nc.tensor.matmul(
                                ps,
                                AT[:, dc, st * P:(st + 1) * P],
                                wo_s[:, dc, eb * 512:(eb + 1) * 512],
                                start=(dc == 0),
                                stop=(dc == 1),
                            )
                        if eb == 0:
                            nc.scalar.activation(
                                ys[:, 0:512], ps, AF.Copy)
                        else:
                            nc.vector.tensor_copy(ys[:, 512:1024], ps)
                    q = nc.sync if st % 2 == 0 else nc.gpsimd
                    q.dma_start(y_d[st * P:(st + 1) * P, :], ys)

            emit_xpose(0)
            for sb in range(N_SB):
                emit_qkv_proj(sb)
                if sb + 1 < N_SB:
                    emit_xpose(sb + 1)
                if sb >= 1:
                    emit_outproj(sb - 1)
                if stop_after <= 1 and sb == 0:
                    break
                emit_attention(sb)
            if stop_after > 1:
                emit_outproj(N_SB - 1)

    return nc


_NC_CACHE = {}


def _get_nc(mm_mode="bf16"):
    if mm_mode not in _NC_CACHE:
        _NC_CACHE[mm_mode] = build_nc(mm_mode=mm_mode)
    return _NC_CACHE[mm_mode]


MM_MODE = "bf16"


def make_in_maps(x, Wq, bq, Wk, bk, Wv, bv, Wo, mm_mode=None):
    """Per-core input dicts: core i -> (batch i//4, head-group i%4)."""
    bf = ml_dtypes.bfloat16
    in_maps = []
    for core in range(8):
        b, g = core // 4, core % 4
        sl = slice(g * DPC, (g + 1) * DPC)
        wqkv = np.concatenate(
            [Wq[:, sl], Wk[:, sl], Wv[:, sl]], axis=0)
        b3 = np.concatenate([bq[sl], bk[sl], bv[sl]])
        in_maps.append({
            "x": np.ascontiguousarray(x[b]).astype(bf),
            "wqkv": np.ascontiguousarray(wqkv).astype(bf),
            "wo": np.ascontiguousarray(Wo[sl, :]).astype(bf),
            "b3": np.ascontiguousarray(b3).astype(np.float32),
        })
    return in_maps


def combine_results(results, bo):
    out = np.zeros((2, S, D), dtype=np.float32)
    for core in range(8):
        out[core // 4] += results[core]["y"]
    out += bo.astype(np.float32)
    return out


_RUNNER_CACHE = {}


def get_runner(mm_mode=None):
    """Build (once) a jitted 8-core runner; returns fn(in_maps) -> results."""
    mode = mm_mode or MM_MODE
    if mode in _RUNNER_CACHE:
        return _RUNNER_CACHE[mode]

    import jax
    from jax.sharding import Mesh, PartitionSpec
    from jax.experimental.shard_map import shard_map
    from concourse import bass2jax, mybir as _mb

    nc = _get_nc(mode)
    bass2jax.install_neuronx_cc_hook()

    pname = nc.partition_id_tensor.name if nc.partition_id_tensor else None
    in_names, out_names, out_avals = [], [], []
    for alloc in nc.m.functions[0].allocations:
        if not isinstance(alloc, _mb.MemoryLocationSet):
            continue
        name = alloc.memorylocations[0].name
        if alloc.kind == "ExternalInput":
            if name != pname:
                in_names.append(name)
        elif alloc.kind == "ExternalOutput":
            out_names.append(name)
            out_avals.append(jax.core.ShapedArray(
                tuple(alloc.tensor_shape), _mb.dt.np(alloc.dtype)))
    n_params = len(in_names)
    all_names = in_names + out_names
    if pname is not None:
        all_names = all_names + [pname]

    def _body(*args):
        operands = list(args)
        if pname is not None:
            operands.append(bass2jax.partition_id_tensor())
        outs = bass2jax._bass_exec_p.bind(
            *operands,
            out_avals=tuple(out_avals),
            in_names=tuple(all_names),
            out_names=tuple(out_names),
            lowering_input_output_aliases=(),
            sim_require_finite=True,
            sim_require_nnan=True,
            nc=nc,
        )
        return tuple(outs)

    devices = jax.devices()[:8]
    mesh = Mesh(np.asarray(devices), ("core",))
    sharded = jax.jit(
        shard_map(_body, mesh=mesh,
                  in_specs=(PartitionSpec("core"),) * (n_params + len(out_names)),
                  out_specs=(PartitionSpec("core"),) * len(out_names),
                  check_rep=False),
        keep_unused=True,
    )

    from jax.sharding import NamedSharding
    zero_outs = [
        jax.device_put(
            np.zeros((8 * a.shape[0], *a.shape[1:]), a.dtype),
            NamedSharding(mesh, PartitionSpec("core")),
        )
        for a in out_avals
    ]

    def run(in_maps):
        concat_in = [
            np.concatenate([np.asarray(m[name]) for m in in_maps], axis=0)
            for name in in_names
        ]
        out_arrs = sharded(*concat_in, *zero_outs)
        return [
            {name: np.asarray(out_arrs[i]).reshape(8, *out_avals[i].shape)[c]
             for i, name in enumerate(out_names)}
            for c in range(8)
        ]

    run.sharded = sharded
    run.in_names = in_names
    run.out_names = out_names
    run.out_avals = out_avals
    run.zero_outs = zero_outs
    _RUNNER_CACHE[mode] = run
    return run


def kernel(x, Wq, bq, Wk, bk, Wv, bv, Wo, bo, **_ignored):
    x = np.asarray(x, dtype=np.float32)
    in_maps = make_in_maps(
        x,
        np.asarray(Wq, np.float32), np.asarray(bq, np.float32),
        np.asarray(Wk, np.float32), np.asarray(bk, np.float32),
        np.asarray(Wv, np.float32), np.asarray(bv, np.float32),
        np.asarray(Wo, np.float32),
    )
    try:
        results = get_runner(MM_MODE)(in_maps)
    except Exception:
        # fallback: stock SPMD runner (slower dispatch, same NEFF)
        from concourse.bass_utils import run_bass_kernel_spmd
        results = run_bass_kernel_spmd(
            _get_nc(MM_MODE), in_maps, core_ids=list(range(8))).results
    return combine_results(results, np.asarray(bo, np.float32))


# revision 32
# speedup vs baseline: 404.4581x; 1.0039x over previous
"""Causal self-attention (D=1024, H=16, S=2048, B=2) on 8 trn2 cores.

Sharding: core i handles batch b = i // 4 and head-group g = i % 4
(4 heads = 256 model dims per group). Each core computes
    y_partial[b,g] = softmax_causal(Q K^T / 8) V  @ Wo[rows of g]
for its 4 heads; the host sums the 4 group partials per batch and adds bo.

Per-core kernel (bf16 matmul operands, fp32 PSUM accumulation), emitted
as a software pipeline over 512-column s-blocks:
  per sb: QT/KT = (Wq/Wk)^T x^T + b (ACT Identity-with-bias psum moves),
          V tiles (DVE bias adds), then attention for both head pairs:
          scoresT = KT^T QT on PE row-group pairs, exp on ACT, triangular
          diag masks on Pool, PV accumulation with the softmax-denominator
          ones-column trick. Head e=0 packs V|1 (denom at psum row 64),
          head e=1 packs 1|V and lands at psum partitions 63..127 so both
          heads normalize lane-aligned: DVE fast-reciprocal of the denom
          row, Pool partition_broadcast, DVE multiply -> AT bf16.
          Output projection y = A Wo trails one s-block behind.
DMA queues: x DMA-transposes split across sync+tensor queues; weights on
gpsimd; y stores on gpsimd/sync (cheap DGE issue paths).
"""

import sys

sys.path.insert(0, "/opt/trn_rl_repo")

import ml_dtypes
import numpy as np

import concourse.bass as bass
import concourse.mybir as mybir
import concourse.tile as tile
from concourse import bacc

P = 128
S = 2048
D = 1024
NH = 4                    # heads per core
DH = 64                   # head dim
DPC = NH * DH             # model dims per core = 256
N_CT = D // P             # 8 contraction chunks
N_ST = S // P             # 16 t tiles of 128
N_SB = S // 512           # 4 s blocks of 512
F32 = mybir.dt.float32
BF16 = mybir.dt.bfloat16
SCALE = 1.0 / 8.0         # 1/sqrt(64)

AF = mybir.ActivationFunctionType
ALU = mybir.AluOpType


def build_nc(mm_mode: str = "bf16", stop_after: int = 99,
             skip_norm: bool = False) -> bass.Bass:
    nc = _build(mm_mode, stop_after, skip_norm)
    if not nc.is_finalized():
        nc.finalize()
    return nc


def _build(mm_mode: str, stop_after: int, skip_norm: bool) -> bass.Bass:
    assert mm_mode == "bf16"
    nc = bacc.Bacc("TRN2", target_bir_lowering=False, debug=False,
                   num_devices=8)

    x_d = nc.dram_tensor("x", [S, D], BF16, kind="ExternalInput")
    # wq/wk/wv stacked host-side: one DMA instead of three
    wqkv_d = nc.dram_tensor("wqkv", [3 * D, DPC], BF16, kind="ExternalInput")
    wo_d = nc.dram_tensor("wo", [DPC, D], BF16, kind="ExternalInput")
    # bq|bk|bv stacked host-side: one DMA instead of three
    b3_d = nc.dram_tensor("b3", [3 * DPC], F32, kind="ExternalInput")
    # bf16 partials: host sums 4 of them per batch in fp32
    y_d = nc.dram_tensor("y", [S, D], BF16, kind="ExternalOutput")

    with tile.TileContext(nc) as tc:
        with (
            tc.tile_pool(name="const", bufs=1) as const,
            tc.tile_pool(name="xtp", bufs=1) as xtp,
            tc.tile_pool(name="qkv", bufs=1) as qkv,
            tc.tile_pool(name="atp", bufs=1) as atp,
            tc.tile_pool(name="exw", bufs=4) as exw,
            tc.tile_pool(name="rcpw", bufs=2) as rcpw,
            tc.tile_pool(name="bcw", bufs=2) as bcw,
            tc.tile_pool(name="ysp", bufs=4) as ysp,
            tc.tile_pool(name="psA", bufs=2, space="PSUM") as psA,
            tc.tile_pool(name="psc", bufs=2, space="PSUM") as psc,
            tc.tile_pool(name="ppv", bufs=2, space="PSUM") as ppv,
        ):
            # ---- weights / constants: 3 DMAs total on the sync HWDGE
            # queue, ahead of the x transposes (the DMA semaphore ring
            # serializes across queues, so every early DMA counts) ----
            # x's first half leads the queue so PE transposes start ASAP
            x_nat = [xtp.tile([P, 8, 1024], BF16, tag=f"xn{h}",
                              name=f"xn{h}") for h in range(2)]
            nc.sync.dma_start(
                x_nat[0],
                x_d[0:1024, :].rearrange("(tt p) d -> p tt d", p=P))
            wqkv_s = const.tile([P, 3 * N_CT, DPC], BF16)
            nc.sync.dma_start(wqkv_s,
                              wqkv_d.rearrange("(o p) d -> p o d", p=P))
            wq_s = wqkv_s[:, 0 * N_CT:1 * N_CT, :]
            wk_s = wqkv_s[:, 1 * N_CT:2 * N_CT, :]
            wv_s = wqkv_s[:, 2 * N_CT:3 * N_CT, :]
            # Wo packed by head pairs: rows 128*dc .. 128*dc+127
            wo_s = const.tile([P, 2, D], BF16)
            nc.sync.dma_start(wo_s, wo_d.rearrange("(dc p) e -> p dc e", p=P))

            # biases as bf16 rows on partition 0; applied by K=1
            # ones-stationary matmuls that seed the projection psums
            b3_f32 = const.tile([1, 3 * DPC], F32)
            nc.sync.dma_start(b3_f32, b3_d[:].unsqueeze(0))
            b3_row = const.tile([1, 3 * DPC], BF16)
            nc.vector.tensor_copy(b3_row, b3_f32)
            bq_row = b3_row[:, 0 * DPC:1 * DPC]
            bk_row = b3_row[:, 1 * DPC:2 * DPC]
            bv_row = b3_row[:, 2 * DPC:3 * DPC]
            # bf16 ones: K=1 stationary / moving rows for broadcasts
            ones_s = const.tile([P, 512], BF16)
            nc.vector.memset(ones_s, 1.0)
            # bf16 identity for PE-transposes of x
            ident = const.tile([P, P], BF16)
            nc.gpsimd.affine_select(
                out=ident, in_=ones_s[:, 0:P], compare_op=ALU.is_equal,
                fill=0.0, base=0, pattern=[[1, P]], channel_multiplier=-1)

            # ---- phase 0: x loaded straight (2 DMAs), transposed on the
            # otherwise-idle PE via identity matmuls. The DMA semaphore
            # ring serializes all queues, so replacing 16 XBAR transposes
            # with 2 plain loads removes ~30us of startup ladder.
            xT = [xtp.tile([P, S], BF16, tag=f"xt{c}", name=f"xt{c}")
                  for c in range(N_CT)]
            nc.sync.dma_start(
                x_nat[1],
                x_d[1024:2048, :].rearrange("(tt p) d -> p tt d", p=P))

            def emit_xpose(g):
                h, q = divmod(g, 2)
                for cp in range(4):  # chunk pairs
                    tp = psA.tile([P, 1024], BF16, tag="psA",
                                  name=f"tp{g}_{cp}")
                    for ci in range(2):
                        c = 2 * cp + ci
                        for B in range(4):
                            tt = q * 4 + B
                            nc.tensor.matmul(
                                tp[:, ci * 512 + B * P:
                                   ci * 512 + (B + 1) * P],
                                x_nat[h][:, tt, c * P:(c + 1) * P],
                                ident, is_transpose=True,
                                start=True, stop=True)
                    for ci in range(2):
                        c = 2 * cp + ci
                        sl = tp[:, ci * 512:(ci + 1) * 512]
                        dst = xT[c][:, g * 512:(g + 1) * 512]
                        if cp % 2 == 0:
                            nc.scalar.activation(dst, sl, AF.Copy)
                        else:
                            nc.vector.tensor_copy(dst, sl)

            # QT/KT: [128 (head-pair d), dc, s]
            QT = qkv.tile([P, 2, S], BF16)
            KT = qkv.tile([P, 2, S], BF16)
            # V_aug: [t-part, t-chunk, head, 65], col 64 == 1.0 so the PV
            # matmul's psum row 64 accumulates the softmax denominator.
            vaug = qkv.tile([P, N_ST, NH, DH + 1], BF16)
            nc.vector.memset(vaug[:, :, :, DH:DH + 1], 1.0)

            # AT packed by head pairs: [128, dc, s]
            AT = atp.tile([P, 2, S], BF16)

            def emit_qkv_proj(sb):
                for dc in range(2):
                    for w_s, b_row, dst in ((wq_s, bq_row, QT),
                                            (wk_s, bk_row, KT)):
                        ps = psA.tile([P, 512], F32, tag="psA")
                        # K=1 ones matmul seeds the psum with the bias
                        nc.tensor.matmul(
                            ps, b_row[:, dc * P:(dc + 1) * P],
                            ones_s[0:1, :], start=True, stop=False)
                        for c in range(N_CT):
                            nc.tensor.matmul(
                                ps,
                                w_s[:, c, dc * P:(dc + 1) * P],
                                xT[c][:, sb * 512:(sb + 1) * 512],
                                start=False,
                                stop=(c == N_CT - 1),
                            )
                        nc.scalar.activation(
                            dst[:, dc, sb * 512:(sb + 1) * 512], ps, AF.Copy)
                for tt in range(4 * sb, 4 * sb + 4):
                    ps = psA.tile([P, 512], F32, tag="psA")
                    pvs = ps[:, 0:DPC]
                    # K=1 ones matmul seeds the psum with bv broadcast
                    nc.tensor.matmul(ps[:, 0:DPC], ones_s[0:1, 0:P], bv_row,
                                     start=True, stop=False)
                    for c in range(N_CT):
                        nc.tensor.matmul(
                            pvs,
                            xT[c][:, tt * P:(tt + 1) * P],
                            wv_s[:, c, :],
                            start=False,
                            stop=(c == N_CT - 1),
                        )
                    nc.vector.tensor_copy(
                        vaug[:, tt, :, 0:DH],
                        pvs.rearrange("p (h u) -> p h u", h=NH))

            def emit_attention(sb):
                # Head pairs (2*dc, 2*dc+1) share each score/exp tile: the two
                # K=64 score matmuls go to PE row-groups 0 and 64 (concurrent).
                t_cnt = 4 * sb + 4
                for dc in range(2):
                    pvt = [ppv.tile([P, 512], F32, tag="pv",
                                    name=f"pv{sb}_{dc}_{e}")
                           for e in range(2)]
                    # both heads: rows 0..63 = values, row 64 = denominator
                    pv_dst = (pvt[0][0:DH + 1, :], pvt[1][0:DH + 1, :])
                    pend = []  # deferred PV emission: (T, ms, ex)
                    for T in range(t_cnt):
                        k = T - 4 * sb
                        ms = 128 * k if k > 0 else 0
                        sc = psc.tile([P, 2, 512], F32, tag="sc")
                        ex = exw.tile([P, 2, 512], BF16, tag="ex")
                        for e in range(2):  # even/odd head of the pair
                            off = DH * e
                            nc.tensor.matmul(
                                sc[:, e, ms:512],
                                KT[off:off + DH, dc, T * P:(T + 1) * P],
                                QT[off:off + DH, dc,
                                   sb * 512 + ms:(sb + 1) * 512],
                                start=True,
                                stop=True,
                            )
                        nc.scalar.activation(
                            ex[:, :, ms:512], sc[:, :, ms:512],
                            AF.Exp, scale=SCALE,
                        )
                        if k >= 0:  # triangular mask on diagonal chunks
                            nc.gpsimd.affine_select(
                                out=ex[:, :, ms:ms + P],
                                in_=ex[:, :, ms:ms + P],
                                compare_op=ALU.is_ge,
                                fill=0.0,
                                base=0,
                                pattern=[[0, 2], [1, P]],
                                channel_multiplier=-1,
                            )
                        pend.append((T, ms, ex))
                        # defer PV two tiles so its exp is long finished
                        # by the time the PE reaches it (no ramp resets)
                        if len(pend) > 2:
                            _emit_pv(dc, pv_dst, pend.pop(0), t_cnt)
                    while pend:
                        _emit_pv(dc, pv_dst, pend.pop(0), t_cnt)

                    if skip_norm:
                        nc.vector.tensor_copy(
                            AT[0:DH, dc, sb * 512:(sb + 1) * 512],
                            pvt[0][0:DH, :])
                        continue
                    # normalize: ACT-copy denom rows to bf16, PE ones-matmul
                    # broadcasts them to 64 psum partitions, one fast-recip
                    # over both heads, DVE multiplies into AT (e=1 via DMA)
                    drow = rcpw.tile([P, 2, 512], BF16, tag="rcp")
                    bcp = psc.tile([P, 2, 512], F32, tag="sc")
                    for e in range(2):
                        nc.scalar.activation(
                            drow[DH:DH + 1, e, :], pvt[e][DH:DH + 1, :],
                            AF.Copy)
                        nc.tensor.matmul(
                            bcp[0:DH, e, :], ones_s[DH:DH + 1, 0:DH],
                            drow[DH:DH + 1, e, :], start=True, stop=True)
                    rcb = bcw.tile([P, 2, 512], F32, tag="bc")
                    nc.vector.reciprocal_approx_fast(
                        out=rcb[0:DH, :, :], in_=bcp[0:DH, :, :])
                    nc.vector.tensor_mul(
                        AT[0:DH, dc, sb * 512:(sb + 1) * 512],
                        pvt[0][0:DH, :], rcb[0:DH, 0, :])
                    att = exw.tile([DH, 512], BF16, tag="att")
                    nc.vector.tensor_mul(att, pvt[1][0:DH, :], rcb[0:DH, 1, :])
                    nc.gpsimd.dma_start(
                        AT[DH:P, dc, sb * 512:(sb + 1) * 512], att)

            def _emit_pv(dc, pv_dst, item, t_cnt):
                T, ms, ex = item
                for e in range(2):
                    h = 2 * dc + e
                    nc.tensor.matmul(
                        pv_dst[e][:, ms:512],
                        vaug[:, T, h, :],
                        ex[:, e, ms:512],
                        start=(T == 0),
                        stop=(T == t_cnt - 1),
                    )

            def emit_outproj(sb):
                for st in range(4 * sb, 4 * sb + 4):
                    ys = ysp.tile([P, 1024], BF16, tag="ys")
                    for eb in range(2):
                        ps = psA.tile([P, 512], F32, tag="psA")
                        for dc in range(2):
                            nc.tensor.matmul(
                                ps,
                                AT[:, dc, st * P:(st + 1) * P],
                                wo_s[:, dc, eb * 512:(eb + 1) * 512],
                                start=(dc == 0),
                                stop=(dc == 1),
                            )
                        if eb == 0:
                            nc.scalar.activation(
                                ys[:, 0:512], ps, AF.Copy)
                        else:
                            nc.vector.tensor_copy(ys[:, 512:1024], ps)
                    q = nc.sync if st % 2 == 0 else nc.gpsimd
                    q.dma_start(y_d[st * P:(st + 1) * P, :], ys)

            emit_xpose(0)
            for sb in range(N_SB):
                emit_qkv_proj(sb)
                if sb + 1 < N_SB:
                    emit_xpose(sb + 1)
                if sb >= 1:
                    emit_outproj(sb - 1)
                if stop_after <= 1 and sb == 0:
                    break
                emit_attention(sb)
            if stop_after > 1:
                emit_outproj(N_SB - 1)

    return nc


_NC_CACHE = {}


def _get_nc(mm_mode="bf16"):
    if mm_mode not in _NC_CACHE:
        _NC_CACHE[mm_mode] = build_nc(mm_mode=mm_mode)
    return _NC_CACHE[mm_mode]


MM_MODE = "bf16"


def make_in_maps(x, Wq, bq, Wk, bk, Wv, bv, Wo, mm_mode=None):
    """Per-core input dicts: core i -> (batch i//4, head-group i%4)."""
    bf = ml_dtypes.bfloat16
    in_maps = []
    for core in range(8):
        b, g = core // 4, core % 4
        sl = slice(g * DPC, (g + 1) * DPC)
        wqkv = np.concatenate(
            [Wq[:, sl], Wk[:, sl], Wv[:, sl]], axis=0)
        b3 = np.concatenate([bq[sl], bk[sl], bv[sl]])
        in_maps.append({
            "x": np.ascontiguousarray(x[b]).astype(bf),
            "wqkv": np.ascontiguousarray(wqkv).astype(bf),
            "wo": np.ascontiguousarray(Wo[sl, :]).astype(bf),
            "b3": np.ascontiguousarray(b3).astype(np.float32),
        })
    return in_maps


def combine_results(results, bo):
    out = np.zeros((2, S, D), dtype=np.float32)
    for core in range(8):
        out[core // 4] += results[core]["y"]
    out += bo.astype(np.float32)
    return out


_RUNNER_CACHE = {}


def get_runner(mm_mode=None):
    """Build (once) a jitted 8-core runner; returns fn(in_maps) -> results."""
    mode = mm_mode or MM_MODE
    if mode in _RUNNER_CACHE:
        return _RUNNER_CACHE[mode]

    import jax
    from jax.sharding import Mesh, PartitionSpec
    from jax.experimental.shard_map import shard_map
    from concourse import bass2jax, mybir as _mb

    nc = _get_nc(mode)
    bass2jax.install_neuronx_cc_hook()

    pname = nc.partition_id_tensor.name if nc.partition_id_tensor else None
    in_names, out_names, out_avals = [], [], []
    for alloc in nc.m.functions[0].allocations:
        if not isinstance(alloc, _mb.MemoryLocationSet):
            continue
        name = alloc.memorylocations[0].name
        if alloc.kind == "ExternalInput":
            if name != pname:
                in_names.append(name)
        elif alloc.kind == "ExternalOutput":
            out_names.append(name)
            out_avals.append(jax.core.ShapedArray(
                tuple(alloc.tensor_shape), _mb.dt.np(alloc.dtype)))
    n_params = len(in_names)
    all_names = in_names + out_names
    if pname is not None:
        all_names = all_names + [pname]

    def _body(*args):
        operands = list(args)
        if pname is not None:
            operands.append(bass2jax.partition_id_tensor())
        outs = bass2jax._bass_exec_p.bind(
            *operands,
            out_avals=tuple(out_avals),
            in_names=tuple(all_names),
            out_names=tuple(out_names),
            lowering_input_output_aliases=(),
            sim_require_finite=True,
            sim_require_nnan=True,
            nc=nc,
        )
        return tuple(outs)

    devices = jax.devices()[:8]
    mesh = Mesh(np.asarray(devices), ("core",))
    sharded = jax.jit(
        shard_map(_body, mesh=mesh,
                  in_specs=(PartitionSpec("core"),) * (n_params + len(out_names)),
                  out_specs=(PartitionSpec("core"),) * len(out_names),
                  check_rep=False),
        keep_unused=True,
    )

    from jax.sharding import NamedSharding
    zero_outs = [
        jax.device_put(
            np.zeros((8 * a.shape[0], *a.shape[1:]), a.dtype),
            NamedSharding(mesh, PartitionSpec("core")),
        )
        for a in out_avals
    ]

    def run(in_maps):
        concat_in = [
            np.concatenate([np.asarray(m[name]) for m in in_maps], axis=0)
            for name in in_names
        ]
        out_arrs = sharded(*concat_in, *zero_outs)
        return [
            {name: np.asarray(out_arrs[i]).reshape(8, *out_avals[i].shape)[c]
             for i, name in enumerate(out_names)}
            for c in range(8)
        ]

    run.sharded = sharded
    run.in_names = in_names
    run.out_names = out_names
    run.out_avals = out_avals
    run.zero_outs = zero_outs
    _RUNNER_CACHE[mode] = run
    return run


def kernel(x, Wq, bq, Wk, bk, Wv, bv, Wo, bo, **_ignored):
    x = np.asarray(x, dtype=np.float32)
    in_maps = make_in_maps(
        x,
        np.asarray(Wq, np.float32), np.asarray(bq, np.float32),
        np.asarray(Wk, np.float32), np.asarray(bk, np.float32),
        np.asarray(Wv, np.float32), np.asarray(bv, np.float32),
        np.asarray(Wo, np.float32),
    )
    try:
        results = get_runner(MM_MODE)(in_maps)
    except Exception:
        # fallback: stock SPMD runner (slower dispatch, same NEFF)
        from concourse.bass_utils import run_bass_kernel_spmd
        results = run_bass_kernel_spmd(
            _get_nc(MM_MODE), in_maps, core_ids=list(range(8))).results
    return combine_results(results, np.asarray(bo, np.float32))
